# revision 45
# baseline (speedup 1.0000x reference)
"""Causal self-attention (B=2, T=2048, D=1024, H=16, Dh=64) on 8 TRN2 cores.

Sharding: core c = 4*b + g -> batch b (data parallel), head group g of 4
heads (tensor parallel on heads for Wq/Wk/Wv, column-split of the proj
input with the resulting partial-sum reduction done host-side at unshard).

Per-core dataflow (layouts chosen so no on-device transposes are needed):
  qT,kT [256, 2048] bf16 = W{q,k}_g @ x.T  (fp8 DoubleRow projections; x8
  is shipped pre-cast fp8 from the host so no on-device casts are needed.
  Note: on HW, DR costs 1 cy/output-col like bf16 -- its win is the 2x
  contraction depth per instruction, NOT a faster column rate.)
  v     [t-block 128, 4 heads x (64 v | 64 ones)] bf16
  attention, transposed: PT[tk, tq] = kT_h.T @ qT_h (bf16), exp on
  ACT -> bf16, causal mask as post-exp 0/1 multiply on GPSIMD,
  AV: yT[d, tq] + softmax column sums free via the ones columns of v
  normalize: yT * recip(sums) -> ytsb [256, 2048] f32r (proj lhsT layout)
  proj partial: out[t, :] = ytsb.T-block @ Wp_gT  (f32r)
Host: out[b] = sum_g partial[4b+g] + bp.

Segment order runs all Ti=0 (first T-half) segs for the 4 heads first, so
the first half of the output projection (and its out-DMA traffic) spreads
over the Ti=1 segs instead of bunching at the tail.
"""

import numpy as np

import concourse.bass as bass
import concourse.mybir as mybir
import concourse.tile as tile
from concourse import bacc
from concourse import bass_utils

F32 = mybir.dt.float32
F32R = mybir.dt.float32r
BF16 = mybir.dt.bfloat16
FP8 = mybir.dt.float8e4
DR = mybir.MatmulPerfMode.DoubleRow
WS = 16.0          # host scale on Wq/Wk (and bq/bk) so fp8 keeps mantissa

B, T, D = 2, 2048, 1024
H, DH = 16, 64
N_CORES = 8
HPC = 4            # heads per core
GD = HPC * DH      # 256 feature cols per core
KT = D // 128      # 8 k-tiles over the model dim
TB = T // 128      # 16 t-blocks of 128
SCL = 0.125        # logit scale 1/sqrt(Dh)

_cache = {}


def _build():
    nc = bacc.Bacc("TRN2", target_bir_lowering=False, debug=False,
                   num_devices=N_CORES)

    xT_d = nc.dram_tensor("xT", [D, T], BF16, kind="ExternalInput")
    # x8: host-cast fp8 copy of x, chunk-major [128, 4, KT, 512] so each
    # 512-col chunk is one contiguous-per-partition DMA
    x8_d = nc.dram_tensor("x8", [128, 4, KT, 512], FP8, kind="ExternalInput")
    wqT_d = nc.dram_tensor("wqT", [128, KT, GD], FP8, kind="ExternalInput")
    wkT_d = nc.dram_tensor("wkT", [128, KT, GD], FP8, kind="ExternalInput")
    wvT_d = nc.dram_tensor("wvT", [128, KT, GD], BF16, kind="ExternalInput")
    wpT_d = nc.dram_tensor("wpT", [GD, D], F32R, kind="ExternalInput")
    bq_d = nc.dram_tensor("bq2", [128, 2], F32, kind="ExternalInput")
    bk_d = nc.dram_tensor("bk2", [128, 2], F32, kind="ExternalInput")
    bvb_d = nc.dram_tensor("bvb", [128, GD], F32, kind="ExternalInput")
    msk_d = nc.dram_tensor("mask01", [128, 128], BF16, kind="ExternalInput")
    out_d = nc.dram_tensor("out", [T, D], BF16, kind="ExternalOutput")
    wrm_d = nc.dram_tensor("wrm", [128, 1], F32, kind="ExternalOutput")

    with tile.TileContext(nc) as tc:
        with (
            tc.tile_pool(name="const", bufs=1) as cp,
            tc.tile_pool(name="big", bufs=1) as bp_,
            tc.tile_pool(name="work", bufs=4) as wp_,
            tc.tile_pool(name="outp", bufs=6) as op_,
            tc.tile_pool(name="pA", bufs=2, space="PSUM") as pA,
            tc.tile_pool(name="pB", bufs=1, space="PSUM") as pB,
            tc.tile_pool(name="pC", bufs=2, space="PSUM") as pC,
        ):
            # ---- loads, ordered by need-time. First matmul needs wq + x8
            # chunk 0 only; queue issue cost (~0.6us each) is the real head
            # constraint, so big tensors ride single DMAs. ----
            wq = cp.tile([128, KT, GD], FP8, tag="wq", name="wq")
            wk = cp.tile([128, KT, GD], FP8, tag="wk", name="wk")
            wv = cp.tile([128, KT, GD], BF16, tag="wv", name="wv")
            bq2 = cp.tile([128, 2], F32, tag="bq2", name="bq2")
            bk2 = cp.tile([128, 2], F32, tag="bk2", name="bk2")
            bvb = cp.tile([128, GD], F32, tag="bvb", name="bvb")
            msk = cp.tile([128, 128], BF16, tag="msk", name="msk")
            x8 = cp.tile([128, 4, KT, 512], FP8, tag="x8", name="x8")
            xt = [cp.tile([128, T], BF16, tag=f"xt{k}", name=f"xt{k}")
                  for k in range(KT)]

            # gpsimd issues no loads: its queue must stay clear for the
            # exp->mask->AV critical chain
            # first matmul needs only wq k-tiles 0:2 + x8 chunk-0 k-tiles
            # 0:2; split those loads so transfer time off the critical path
            nc.scalar.dma_start(wq[:, 0:2], wqT_d[:, 0:2])
            nc.sync.dma_start(x8[:, 0, 0:2], x8_d[:, 0, 0:2])
            nc.scalar.dma_start(wq[:, 2:4], wqT_d[:, 2:4])
            nc.sync.dma_start(x8[:, 0, 2:4], x8_d[:, 0, 2:4])
            nc.scalar.dma_start(wq[:, 4:KT], wqT_d[:, 4:KT])
            nc.sync.dma_start(x8[:, 0, 4:KT], x8_d[:, 0, 4:KT])
            nc.scalar.dma_start(bq2[:], bq_d[:])
            # dummy exp during the head: pulls the ACT table load (~1.3us)
            # off the first attention block's critical path
            wrmup = cp.tile([128, 2], F32, tag="wrmup", name="wrmup")
            nc.scalar.activation(wrmup[:], bq2[:],
                                 mybir.ActivationFunctionType.Exp,
                                 scale=1.0)
            nc.sync.dma_start(wk[:], wkT_d[:])
            nc.scalar.dma_start(bk2[:], bk_d[:])
            nc.sync.dma_start(x8[:, 1, 0:4], x8_d[:, 1, 0:4])
            nc.sync.dma_start(x8[:, 1, 4:KT], x8_d[:, 1, 4:KT])
            nc.scalar.dma_start(x8[:, 2], x8_d[:, 2])
            nc.scalar.dma_start(x8[:, 3], x8_d[:, 3])
            nc.sync.dma_start(msk[:], msk_d[:])
            nc.sync.dma_start(wv[:], wvT_d[:])
            nc.sync.dma_start(bvb[:], bvb_d[:])
            # xt rows: needed by v_groups (first ~12us in); one DMA per row
            for k in range(KT):
                (nc.sync if k % 2 == 0 else nc.scalar).dma_start(
                    xt[k][:], xT_d[k * 128:(k + 1) * 128, :])
            wpt = []
            for p in range(2):
                t_ = cp.tile([128, D], F32R, tag=f"wp{p}", name=f"wp{p}")
                nc.scalar.dma_start(t_[:], wpT_d[p * 128:(p + 1) * 128, :])
                wpt.append(t_)

            qt = [bp_.tile([128, T], BF16, tag=f"qt{m}", name=f"qt{m}")
                  for m in range(2)]
            kt = [bp_.tile([128, T], BF16, tag=f"kt{m}", name=f"kt{m}")
                  for m in range(2)]
            ytsb = [bp_.tile([128, T], F32R, tag=f"yt{p}", name=f"yt{p}")
                    for p in range(2)]
            vt = [bp_.tile([128, 4, 2, DH], BF16, tag=f"v{t}", name=f"v{t}")
                  for t in range(TB)]

            def qk_group(dst, w, b2, m, n):
                # fp8 DoubleRow: two k-tiles per matmul (lhsT [128,2,128],
                # rhs [128,2,512]) -- on HW, DR costs 1 cy per output col
                # like bf16, but packs 2x contraction depth per instruction,
                # so 4 full-width matmuls cover all 8 k-tiles.
                # Values carry a x16 host scale; exp scale compensates.
                ps = pC.tile([128, 512], F32, tag=pC.name, name="psqk")
                for kp in range(4):
                    nc.tensor.matmul(
                        ps[:],
                        w[:, 2 * kp:2 * kp + 2, m * 128:(m + 1) * 128],
                        x8[:, n, 2 * kp:2 * kp + 2, :],
                        start=(kp == 0), stop=(kp == 3),
                        perf_mode=DR,
                    )
                nc.vector.tensor_scalar_add(
                    dst[m][:, n * 512:(n + 1) * 512], ps[:], b2[:, m:m + 1],
                )

            def v_group(t):
                ps = pC.tile([128, 512], F32, tag=pC.name, name="psv")
                for k in range(KT):
                    nc.tensor.matmul(
                        ps[:, 0:GD],
                        xt[k][:, t * 128:(t + 1) * 128],
                        wv[:, k, :],
                        start=(k == 0), stop=(k == KT - 1),
                    )
                nc.vector.tensor_add(
                    vt[t][:, :, 0, :],
                    ps[:, 0:GD].rearrange("p (h d) -> p h d", h=4),
                    bvb.rearrange("p (h d) -> p h d", h=4),
                )

            def proj_group(t, copy_eng):
                # each 512-col half is copied then DMA'd out immediately so
                # the final-output tail pipelines at half-tile granularity.
                # Late groups (copy_eng == "act") run during the tail: the
                # PSUM->SBUF copies go to the then-idle scalar engine, and
                # the out-DMAs split across queues so the last transfers
                # ride several DMA engines instead of one.
                ob = op_.tile([128, 1024], BF16, tag="ob", name="ob")
                for n in range(2):
                    po = pC.tile([128, 512], F32, tag=pC.name, name="pso")
                    for p in range(2):
                        nc.tensor.matmul(
                            po[:],
                            ytsb[p][:, 128 * t:128 * (t + 1)],
                            wpt[p][:, 512 * n:512 * (n + 1)],
                            start=(p == 0), stop=(p == 1),
                        )
                    if copy_eng == "act":
                        nc.scalar.copy(ob[:, 512 * n:512 * (n + 1)], po[:])
                        for h_ in range(2):
                            cs = slice(512 * n + 256 * h_,
                                       512 * n + 256 * (h_ + 1))
                            eng = (nc.sync, nc.scalar,
                                   nc.gpsimd)[(2 * t + 2 * n + h_) % 3]
                            eng.dma_start(
                                out_d[128 * t:128 * (t + 1), cs], ob[:, cs])
                    else:
                        nc.vector.tensor_copy(
                            ob[:, 512 * n:512 * (n + 1)], po[:])
                        nc.sync.dma_start(
                            out_d[128 * t:128 * (t + 1),
                                  512 * n:512 * (n + 1)],
                            ob[:, 512 * n:512 * (n + 1)])

            def attention_seg(Ti, h, fillers, every, last=False,
                              post=None):
                hp, j = h // 2, h % 2
                ytp = pB.tile([128, 1024], F32, tag=pB.name, name="psyt")
                nblk = 8 * (Ti + 1)
                SKEW = 3       # AV trails QK/exp: the PE never
                pend = []      # waits on an exp that was just issued
                def do_av(tkb, ptsb):
                    s = max(0, 128 * tkb - 1024 * Ti)
                    for bk in range(2):
                        c0, c1 = max(s, 512 * bk), 512 * (bk + 1)
                        if c0 >= c1:
                            continue
                        nc.tensor.matmul(
                            ytp[:, c0:c1],
                            vt[tkb][:, h, :, :].rearrange("p a d -> p (a d)"),
                            ptsb[:, c0:c1],
                            start=(tkb == 0), stop=(tkb == nblk - 1),
                        )
                for tkb in range(nblk + SKEW):
                    if tkb < nblk:
                        s = max(0, 128 * tkb - 1024 * Ti)
                        pt = pA.tile([128, 1024], F32, tag=pA.name,
                                     name="pspt")
                        for bk in range(2):
                            c0, c1 = max(s, 512 * bk), 512 * (bk + 1)
                            if c0 >= c1:
                                continue
                            nc.tensor.matmul(
                                pt[:, c0:c1],
                                kt[hp][64 * j:64 * j + 64,
                                       128 * tkb:128 * (tkb + 1)],
                                qt[hp][64 * j:64 * j + 64,
                                       1024 * Ti + c0:1024 * Ti + c1],
                                start=True, stop=True,
                            )
                        ptsb = wp_.tile([128, 1024], BF16, tag="ptsb",
                                        name="ptsb", bufs=6)
                        nc.scalar.activation(
                            ptsb[:, s:1024], pt[:, s:1024],
                            mybir.ActivationFunctionType.Exp,
                            scale=SCL / (WS * WS),
                        )
                        if 128 * tkb >= 1024 * Ti:  # diagonal block
                            nc.gpsimd.tensor_mul(
                                ptsb[:, s:s + 128], ptsb[:, s:s + 128],
                                msk[:],
                            )
                        pend.append((tkb, ptsb))
                    if tkb >= SKEW:
                        do_av(*pend.pop(0))
                    if fillers and tkb % every == every - 1:
                        fillers.pop(0)()
                while pend:
                    do_av(*pend.pop(0))
                # free the PSUM accumulator promptly; 1/sums via the custom
                # fast-approx DVE reciprocal (~5x cheaper than InstReciprocal,
                # 18 good bits -- plenty for softmax normalization)
                if not last:
                    src = wp_.tile([128, 1024], F32, tag="ysb", name="ysb")
                    nc.vector.tensor_copy(src[:], ytp[:])
                else:
                    src = ytp
                # custom-DVE fast reciprocal: in/out must share a partition
                # base (offset-crossing APs feed it garbage -> NaN), and
                # tensor_tensor needs equal input bases -- so recip at base
                # 64, then a single-input gpsimd copy crosses down to base 0
                rc = wp_.tile([128, 1024], F32, tag="recip", name="recip")
                cp_eng = nc.gpsimd if last else nc.vector
                for q in range(4):
                    cs = slice(256 * q, 256 * (q + 1))
                    nc.vector.reciprocal_approx_fast(rc[:, cs], src[:, cs])
                    cp_eng.tensor_copy(rc[0:64, cs], rc[64:128, cs])
                    nc.vector.tensor_mul(
                        ytsb[hp][64 * j:64 * j + 64,
                                 1024 * Ti + 256 * q:1024 * Ti + 256 * (q + 1)],
                        src[0:64, cs], rc[0:64, cs],
                    )
                    if post:
                        post.pop(0)()
                        post.pop(0)()

            # ---- schedule ----
            # all ones-column memsets up front: a memset inside v_group
            # lands on the gpsimd queue behind mask-multiplies that wait on
            # exp, chaining the v fillers (and the PE) to the exp pace.
            # vt[0] gets BOTH regions memset so the clock pre-ramp below can
            # use it as a fully-defined operand with no DMA dependency.
            nc.gpsimd.memset(vt[0][:, :, 0, :], 1.0)
            for t in range(TB):
                nc.gpsimd.memset(vt[t][:, :, 1, :], 1.0)
            # head clock pre-ramp: the PE idles ~10us while loads land, then
            # pays ~3.5us of half-clock p-state ramp on its first real
            # matmuls. Burn dummy matmuls (on the memset-only vt[0], ready
            # within ~1us) through the idle window so the real work starts
            # at full clock. reduce_max into a debug output prevents
            # elimination; run() ignores it.
            wrm = bp_.tile([128, 1], F32, tag="wrm", name="wrm")
            wps = pC.tile([128, 512], F32, tag=pC.name, name="pswrm")
            NWARM = 10
            for i in range(NWARM):
                nc.tensor.matmul(
                    wps[:],
                    vt[0][:, 0, :, :].rearrange("p a d -> p (a d)"),
                    vt[0].rearrange("p h a d -> p (h a d)"),
                    start=(i == 0), stop=(i == NWARM - 1),
                )
            nc.vector.reduce_max(wrm[:], wps[:], axis=mybir.AxisListType.X)
            # q/k projections for the first T-half (n=0,1) of BOTH m-chunks
            # up front: every Ti=0 seg only touches q/k columns 0:1024.
            # The three head-start v groups (seg 0's SKEW lead) sit between
            # the n=0 and n=1 groups to cover the x8 chunk-1 transfer.
            for m in range(2):
                qk_group(qt, wq, bq2, m, 0)
                qk_group(kt, wk, bk2, m, 0)
            for t in range(3):
                v_group(t)
            for m in range(2):
                qk_group(qt, wq, bq2, m, 1)
                qk_group(kt, wk, bk2, m, 1)

            def f_qk(n):  # remaining projection column chunks
                return [lambda m=m, w=w, b=b, d=d: qk_group(d, w, b, m, n)
                        for m in range(2) for (d, w, b) in
                        [(qt, wq, bq2), (kt, wk, bk2)]]
            f_v03 = [lambda t=t: v_group(t) for t in range(3, 8)]
            f_v = [lambda t=t: v_group(t) for t in range(8, 16)]
            f_p0 = [lambda t=t: proj_group(t, "vec") for t in range(8)]

            n2 = f_qk(2)
            # Ti=0 for all heads first, then Ti=1; the first output half's
            # proj groups drip through the Ti=1 segs
            attention_seg(0, 0, [n2[0], f_v03[0], n2[1], f_v03[1], n2[2],
                                 f_v03[2], n2[3], f_v03[3], f_v03[4]], 1)
            attention_seg(0, 1, f_qk(3), 2)
            attention_seg(0, 2, f_v[0:4], 2)
            attention_seg(0, 3, f_v[4:8], 2)
            attention_seg(1, 0, f_p0[0:3], 4)
            attention_seg(1, 1, f_p0[3:6], 4)
            attention_seg(1, 2, f_p0[6:8], 8)
            # proj for the second T-half drips into the last seg's normalize
            # quarters (each 256-col quarter unlocks two 128-row t-blocks)
            f_p1 = [lambda t=t: proj_group(t, "act") for t in range(8, 16)]
            attention_seg(1, 3, [], 4, last=True, post=f_p1)
            nc.sync.dma_start(wrm_d[:], wrm[:])

    nc.compile()
    return nc


def _shard(x, Wq, bq, Wk, bk, Wv, bv, Wp, bp):
    import ml_dtypes
    f32 = np.float32
    bf16 = ml_dtypes.bfloat16
    fp8 = ml_dtypes.float8_e4m3fn
    mask01 = np.triu(np.ones((128, 128), f32)).astype(bf16)

    def ptile(w):  # [D, GD] -> [128, KT, GD] partition-contiguous
        return np.ascontiguousarray(
            w.reshape(KT, 128, GD).transpose(1, 0, 2))

    in_maps = []
    for c in range(N_CORES):
        b, g = divmod(c, HPC)
        sl = slice(GD * g, GD * (g + 1))
        xTb = np.ascontiguousarray(x[b].T)
        # [D, T] -> [128, 4, KT, 512] chunk-major fp8
        x8b = np.ascontiguousarray(
            xTb.astype(fp8).reshape(KT, 128, 4, 512).transpose(1, 2, 0, 3))
        in_maps.append({
            "xT": xTb.astype(bf16),
            "x8": x8b,
            "wqT": ptile((WS * Wq[sl, :].T).astype(fp8)),
            "wkT": ptile((WS * Wk[sl, :].T).astype(fp8)),
            "wvT": ptile(Wv[sl, :].T.astype(bf16)),
            "wpT": np.ascontiguousarray(Wp[:, sl].T, dtype=f32),
            "bq2": np.ascontiguousarray(
                WS * bq[sl].reshape(2, 128).T, dtype=f32),
            "bk2": np.ascontiguousarray(
                WS * bk[sl].reshape(2, 128).T, dtype=f32),
            "bvb": np.broadcast_to(bv[sl], (128, GD)).astype(f32),
            "mask01": mask01,
        })
    return in_maps


def run(inputs, trace=False):
    """Run the SPMD kernel; returns (output [B,T,D] f32, BassKernelResults)."""
    if "nc" not in _cache:
        _cache["nc"] = _build()
    nc = _cache["nc"]
    in_maps = _shard(**inputs)
    if trace:
        _install_ntff_hook()
    res = bass_utils.run_bass_kernel_spmd(
        nc, in_maps, core_ids=list(range(N_CORES)), trace=trace,
    )
    bp = np.asarray(inputs["bp"], dtype=np.float32)
    out = np.empty((B, T, D), dtype=np.float32)
    for b in range(B):
        acc = res.results[4 * b]["out"].astype(np.float32)
        for g in range(1, HPC):
            acc = acc + res.results[4 * b + g]["out"]
        out[b] = acc + bp
    return out, res


def kernel(**inputs):
    out, _ = run(inputs, trace=False)
    return out


def _install_ntff_hook():
    """antenv.axon_hooks is absent on this image; inject it so
    run_bass_kernel_spmd(trace=True) can capture NTFF profiles."""
    import sys, types
    if "antenv.axon_hooks" in sys.modules:
        return
    try:
        mod = types.ModuleType("antenv.axon_hooks")
        mod._hook = None
        mod.set_axon_ntff_profile_hook = lambda h: setattr(mod, "_hook", h)
        mod.get_axon_ntff_profile_hook = lambda: mod._hook
        sys.modules["antenv.axon_hooks"] = mod
        import antenv
        antenv.axon_hooks = mod
        from trn_agent_boot.trn_boot import _ntff_profile_via_ctypes
        mod.set_axon_ntff_profile_hook(
            _ntff_profile_via_ctypes("/opt/axon/libaxon_pjrt.so"))
    except Exception:
        pass


# revision 46
# speedup vs baseline: 1.0425x; 1.0425x over previous
"""Causal self-attention (B=2, T=2048, D=1024, H=16, Dh=64) on 8 TRN2 cores.

Sharding: core c = 4*b + g -> batch b (data parallel), head group g of 4
heads (tensor parallel on heads for Wq/Wk/Wv, column-split of the proj
input with the resulting partial-sum reduction done host-side at unshard).

Per-core dataflow (layouts chosen so no on-device transposes are needed):
  qT,kT [256, 2048] bf16 = W{q,k}_g @ x.T  (fp8 DoubleRow projections; x8
  is shipped pre-cast fp8 from the host so no on-device casts are needed.
  Note: on HW, DR costs 1 cy/output-col like bf16 -- its win is the 2x
  contraction depth per instruction, NOT a faster column rate.)
  v     [t-block 128, 4 heads x (64 v | 64 ones)] bf16
  attention, transposed: PT[tk, tq] = kT_h.T @ qT_h (bf16), exp on
  ACT -> bf16, causal mask as post-exp 0/1 multiply on GPSIMD,
  AV: yT[d, tq] + softmax column sums free via the ones columns of v
  normalize: yT * recip(sums) -> ytsb [256, 2048] f32r (proj lhsT layout)
  proj partial: out[t, :] = ytsb.T-block @ Wp_gT  (f32r)
Host: out[b] = sum_g partial[4b+g] + bp.

Segment order runs all Ti=0 (first T-half) segs for the 4 heads first, so
the first half of the output projection (and its out-DMA traffic) spreads
over the Ti=1 segs instead of bunching at the tail.
"""

import numpy as np

import concourse.bass as bass
import concourse.mybir as mybir
import concourse.tile as tile
from concourse import bacc
from concourse import bass_utils

F32 = mybir.dt.float32
F32R = mybir.dt.float32r
BF16 = mybir.dt.bfloat16
FP8 = mybir.dt.float8e4
DR = mybir.MatmulPerfMode.DoubleRow
WS = 16.0          # host scale on Wq/Wk (and bq/bk) so fp8 keeps mantissa

B, T, D = 2, 2048, 1024
H, DH = 16, 64
N_CORES = 8
HPC = 4            # heads per core
GD = HPC * DH      # 256 feature cols per core
KT = D // 128      # 8 k-tiles over the model dim
TB = T // 128      # 16 t-blocks of 128
SCL = 0.125        # logit scale 1/sqrt(Dh)

_cache = {}


def _build():
    nc = bacc.Bacc("TRN2", target_bir_lowering=False, debug=False,
                   num_devices=N_CORES)

    xT_d = nc.dram_tensor("xT", [D, T], BF16, kind="ExternalInput")
    # x8: host-cast fp8 copy of x, chunk-major [128, 4, KT, 512] so each
    # 512-col chunk is one contiguous-per-partition DMA
    x8_d = nc.dram_tensor("x8", [128, 4, KT, 512], FP8, kind="ExternalInput")
    wqT_d = nc.dram_tensor("wqT", [128, KT, GD], FP8, kind="ExternalInput")
    wkT_d = nc.dram_tensor("wkT", [128, KT, GD], FP8, kind="ExternalInput")
    wvT_d = nc.dram_tensor("wvT", [128, KT, GD], BF16, kind="ExternalInput")
    wpT_d = nc.dram_tensor("wpT", [GD, D], F32R, kind="ExternalInput")
    bq_d = nc.dram_tensor("bq2", [128, 2], F32, kind="ExternalInput")
    bk_d = nc.dram_tensor("bk2", [128, 2], F32, kind="ExternalInput")
    bvb_d = nc.dram_tensor("bvb", [128, GD], F32, kind="ExternalInput")
    msk_d = nc.dram_tensor("mask01", [128, 128], BF16, kind="ExternalInput")
    out_d = nc.dram_tensor("out", [T, D], BF16, kind="ExternalOutput")

    with tile.TileContext(nc) as tc:
        with (
            tc.tile_pool(name="const", bufs=1) as cp,
            tc.tile_pool(name="big", bufs=1) as bp_,
            tc.tile_pool(name="work", bufs=4) as wp_,
            tc.tile_pool(name="outp", bufs=6) as op_,
            tc.tile_pool(name="pA", bufs=2, space="PSUM") as pA,
            tc.tile_pool(name="pB", bufs=1, space="PSUM") as pB,
            tc.tile_pool(name="pC", bufs=2, space="PSUM") as pC,
        ):
            # ---- loads, ordered by need-time. First matmul needs wq + x8
            # chunk 0 only; queue issue cost (~0.6us each) is the real head
            # constraint, so big tensors ride single DMAs. ----
            wq = cp.tile([128, KT, GD], FP8, tag="wq", name="wq")
            wk = cp.tile([128, KT, GD], FP8, tag="wk", name="wk")
            wv = cp.tile([128, KT, GD], BF16, tag="wv", name="wv")
            bq2 = cp.tile([128, 2], F32, tag="bq2", name="bq2")
            bk2 = cp.tile([128, 2], F32, tag="bk2", name="bk2")
            bvb = cp.tile([128, GD], F32, tag="bvb", name="bvb")
            msk = cp.tile([128, 128], BF16, tag="msk", name="msk")
            x8 = cp.tile([128, 4, KT, 512], FP8, tag="x8", name="x8")
            xt = [cp.tile([128, T], BF16, tag=f"xt{k}", name=f"xt{k}")
                  for k in range(KT)]

            # gpsimd issues no loads: its queue must stay clear for the
            # exp->mask->AV critical chain
            # first matmul needs only wq k-tiles 0:2 + x8 chunk-0 k-tiles
            # 0:2; split those loads so transfer time off the critical path
            nc.scalar.dma_start(wq[:, 0:2], wqT_d[:, 0:2])
            nc.sync.dma_start(x8[:, 0, 0:2], x8_d[:, 0, 0:2])
            nc.scalar.dma_start(wq[:, 2:4], wqT_d[:, 2:4])
            nc.sync.dma_start(x8[:, 0, 2:4], x8_d[:, 0, 2:4])
            nc.scalar.dma_start(wq[:, 4:KT], wqT_d[:, 4:KT])
            nc.sync.dma_start(x8[:, 0, 4:KT], x8_d[:, 0, 4:KT])
            nc.scalar.dma_start(bq2[:], bq_d[:])
            # dummy exp during the head: pulls the ACT table load (~1.3us)
            # off the first attention block's critical path
            wrmup = cp.tile([128, 2], F32, tag="wrmup", name="wrmup")
            nc.scalar.activation(wrmup[:], bq2[:],
                                 mybir.ActivationFunctionType.Exp,
                                 scale=1.0)
            nc.sync.dma_start(wk[:], wkT_d[:])
            nc.scalar.dma_start(bk2[:], bk_d[:])
            nc.sync.dma_start(x8[:, 1, 0:4], x8_d[:, 1, 0:4])
            nc.sync.dma_start(x8[:, 1, 4:KT], x8_d[:, 1, 4:KT])
            nc.scalar.dma_start(x8[:, 2], x8_d[:, 2])
            nc.scalar.dma_start(x8[:, 3], x8_d[:, 3])
            nc.sync.dma_start(msk[:], msk_d[:])
            nc.sync.dma_start(wv[:], wvT_d[:])
            nc.sync.dma_start(bvb[:], bvb_d[:])
            # xt rows: needed by v_groups (first ~12us in); one DMA per row
            for k in range(KT):
                (nc.sync if k % 2 == 0 else nc.scalar).dma_start(
                    xt[k][:], xT_d[k * 128:(k + 1) * 128, :])
            wpt = []
            for p in range(2):
                t_ = cp.tile([128, D], F32R, tag=f"wp{p}", name=f"wp{p}")
                nc.scalar.dma_start(t_[:], wpT_d[p * 128:(p + 1) * 128, :])
                wpt.append(t_)

            qt = [bp_.tile([128, T], BF16, tag=f"qt{m}", name=f"qt{m}")
                  for m in range(2)]
            kt = [bp_.tile([128, T], BF16, tag=f"kt{m}", name=f"kt{m}")
                  for m in range(2)]
            ytsb = [bp_.tile([128, T], F32R, tag=f"yt{p}", name=f"yt{p}")
                    for p in range(2)]
            vt = [bp_.tile([128, 4, 2, DH], BF16, tag=f"v{t}", name=f"v{t}")
                  for t in range(TB)]

            def qk_group(dst, w, b2, m, n):
                # fp8 DoubleRow: two k-tiles per matmul (lhsT [128,2,128],
                # rhs [128,2,512]) -- on HW, DR costs 1 cy per output col
                # like bf16, but packs 2x contraction depth per instruction,
                # so 4 full-width matmuls cover all 8 k-tiles.
                # Values carry a x16 host scale; exp scale compensates.
                ps = pC.tile([128, 512], F32, tag=pC.name, name="psqk")
                for kp in range(4):
                    nc.tensor.matmul(
                        ps[:],
                        w[:, 2 * kp:2 * kp + 2, m * 128:(m + 1) * 128],
                        x8[:, n, 2 * kp:2 * kp + 2, :],
                        start=(kp == 0), stop=(kp == 3),
                        perf_mode=DR,
                    )
                nc.vector.tensor_scalar_add(
                    dst[m][:, n * 512:(n + 1) * 512], ps[:], b2[:, m:m + 1],
                )

            def v_group(t):
                ps = pC.tile([128, 512], F32, tag=pC.name, name="psv")
                for k in range(KT):
                    nc.tensor.matmul(
                        ps[:, 0:GD],
                        xt[k][:, t * 128:(t + 1) * 128],
                        wv[:, k, :],
                        start=(k == 0), stop=(k == KT - 1),
                    )
                nc.vector.tensor_add(
                    vt[t][:, :, 0, :],
                    ps[:, 0:GD].rearrange("p (h d) -> p h d", h=4),
                    bvb.rearrange("p (h d) -> p h d", h=4),
                )

            def proj_group(t, copy_eng):
                # each 512-col half is copied then DMA'd out immediately so
                # the final-output tail pipelines at half-tile granularity.
                # Late groups (copy_eng == "act") run during the tail: the
                # PSUM->SBUF copies go to the then-idle scalar engine, and
                # the out-DMAs split across queues so the last transfers
                # ride several DMA engines instead of one.
                ob = op_.tile([128, 1024], BF16, tag="ob", name="ob")
                for n in range(2):
                    po = pC.tile([128, 512], F32, tag=pC.name, name="pso")
                    for p in range(2):
                        nc.tensor.matmul(
                            po[:],
                            ytsb[p][:, 128 * t:128 * (t + 1)],
                            wpt[p][:, 512 * n:512 * (n + 1)],
                            start=(p == 0), stop=(p == 1),
                        )
                    if copy_eng == "act":
                        nc.scalar.copy(ob[:, 512 * n:512 * (n + 1)], po[:])
                        for h_ in range(2):
                            cs = slice(512 * n + 256 * h_,
                                       512 * n + 256 * (h_ + 1))
                            eng = (nc.sync, nc.scalar,
                                   nc.gpsimd)[(2 * t + 2 * n + h_) % 3]
                            eng.dma_start(
                                out_d[128 * t:128 * (t + 1), cs], ob[:, cs])
                    else:
                        nc.vector.tensor_copy(
                            ob[:, 512 * n:512 * (n + 1)], po[:])
                        nc.sync.dma_start(
                            out_d[128 * t:128 * (t + 1),
                                  512 * n:512 * (n + 1)],
                            ob[:, 512 * n:512 * (n + 1)])

            def attention_seg(Ti, h, fillers, every, last=False,
                              post=None):
                hp, j = h // 2, h % 2
                ytp = pB.tile([128, 1024], F32, tag=pB.name, name="psyt")
                nblk = 8 * (Ti + 1)
                SKEW = 3       # AV trails QK/exp: the PE never
                pend = []      # waits on an exp that was just issued
                def do_av(tkb, ptsb):
                    s = max(0, 128 * tkb - 1024 * Ti)
                    for bk in range(2):
                        c0, c1 = max(s, 512 * bk), 512 * (bk + 1)
                        if c0 >= c1:
                            continue
                        nc.tensor.matmul(
                            ytp[:, c0:c1],
                            vt[tkb][:, h, :, :].rearrange("p a d -> p (a d)"),
                            ptsb[:, c0:c1],
                            start=(tkb == 0), stop=(tkb == nblk - 1),
                        )
                for tkb in range(nblk + SKEW):
                    if tkb < nblk:
                        s = max(0, 128 * tkb - 1024 * Ti)
                        pt = pA.tile([128, 1024], F32, tag=pA.name,
                                     name="pspt")
                        for bk in range(2):
                            c0, c1 = max(s, 512 * bk), 512 * (bk + 1)
                            if c0 >= c1:
                                continue
                            nc.tensor.matmul(
                                pt[:, c0:c1],
                                kt[hp][64 * j:64 * j + 64,
                                       128 * tkb:128 * (tkb + 1)],
                                qt[hp][64 * j:64 * j + 64,
                                       1024 * Ti + c0:1024 * Ti + c1],
                                start=True, stop=True,
                            )
                        ptsb = wp_.tile([128, 1024], BF16, tag="ptsb",
                                        name="ptsb", bufs=6)
                        nc.scalar.activation(
                            ptsb[:, s:1024], pt[:, s:1024],
                            mybir.ActivationFunctionType.Exp,
                            scale=SCL / (WS * WS),
                        )
                        if 128 * tkb >= 1024 * Ti:  # diagonal block
                            nc.gpsimd.tensor_mul(
                                ptsb[:, s:s + 128], ptsb[:, s:s + 128],
                                msk[:],
                            )
                        pend.append((tkb, ptsb))
                    if tkb >= SKEW:
                        do_av(*pend.pop(0))
                    if fillers and tkb % every == every - 1:
                        fillers.pop(0)()
                while pend:
                    do_av(*pend.pop(0))
                # free the PSUM accumulator promptly; 1/sums via the custom
                # fast-approx DVE reciprocal (~5x cheaper than InstReciprocal,
                # 18 good bits -- plenty for softmax normalization)
                if not last:
                    src = wp_.tile([128, 1024], F32, tag="ysb", name="ysb")
                    nc.vector.tensor_copy(src[:], ytp[:])
                else:
                    src = ytp
                # custom-DVE fast reciprocal: in/out must share a partition
                # base (offset-crossing APs feed it garbage -> NaN), and
                # tensor_tensor needs equal input bases -- so recip at base
                # 64, then a single-input gpsimd copy crosses down to base 0
                rc = wp_.tile([128, 1024], F32, tag="recip", name="recip")
                cp_eng = nc.gpsimd if last else nc.vector
                for q in range(4):
                    cs = slice(256 * q, 256 * (q + 1))
                    nc.vector.reciprocal_approx_fast(rc[:, cs], src[:, cs])
                    cp_eng.tensor_copy(rc[0:64, cs], rc[64:128, cs])
                    nc.vector.tensor_mul(
                        ytsb[hp][64 * j:64 * j + 64,
                                 1024 * Ti + 256 * q:1024 * Ti + 256 * (q + 1)],
                        src[0:64, cs], rc[0:64, cs],
                    )
                    if post:
                        post.pop(0)()
                        post.pop(0)()

            # ---- schedule ----
            # all ones-column memsets up front: a memset inside v_group
            # lands on the gpsimd queue behind mask-multiplies that wait on
            # exp, chaining the v fillers (and the PE) to the exp pace
            for t in range(TB):
                nc.gpsimd.memset(vt[t][:, :, 1, :], 1.0)
            # q/k projections for the first T-half (n=0,1) of BOTH m-chunks
            # up front: every Ti=0 seg only touches q/k columns 0:1024.
            # The three head-start v groups (seg 0's SKEW lead) sit between
            # the n=0 and n=1 groups to cover the x8 chunk-1 transfer.
            for m in range(2):
                qk_group(qt, wq, bq2, m, 0)
                qk_group(kt, wk, bk2, m, 0)
            for t in range(3):
                v_group(t)
            for m in range(2):
                qk_group(qt, wq, bq2, m, 1)
                qk_group(kt, wk, bk2, m, 1)

            def f_qk(n):  # remaining projection column chunks
                return [lambda m=m, w=w, b=b, d=d: qk_group(d, w, b, m, n)
                        for m in range(2) for (d, w, b) in
                        [(qt, wq, bq2), (kt, wk, bk2)]]
            f_v03 = [lambda t=t: v_group(t) for t in range(3, 8)]
            f_v = [lambda t=t: v_group(t) for t in range(8, 16)]
            f_p0 = [lambda t=t: proj_group(t, "vec") for t in range(8)]

            n2 = f_qk(2)
            # Ti=0 for all heads first, then Ti=1; the first output half's
            # proj groups drip through the Ti=1 segs
            attention_seg(0, 0, [n2[0], f_v03[0], n2[1], f_v03[1], n2[2],
                                 f_v03[2], n2[3], f_v03[3], f_v03[4]], 1)
            attention_seg(0, 1, f_qk(3), 2)
            attention_seg(0, 2, f_v[0:4], 2)
            attention_seg(0, 3, f_v[4:8], 2)
            attention_seg(1, 0, f_p0[0:3], 4)
            attention_seg(1, 1, f_p0[3:6], 4)
            attention_seg(1, 2, f_p0[6:8], 8)
            # proj for the second T-half drips into the last seg's normalize
            # quarters (each 256-col quarter unlocks two 128-row t-blocks)
            f_p1 = [lambda t=t: proj_group(t, "act") for t in range(8, 16)]
            attention_seg(1, 3, [], 4, last=True, post=f_p1)

    nc.compile()
    return nc


def _shard(x, Wq, bq, Wk, bk, Wv, bv, Wp, bp):
    import ml_dtypes
    f32 = np.float32
    bf16 = ml_dtypes.bfloat16
    fp8 = ml_dtypes.float8_e4m3fn
    mask01 = np.triu(np.ones((128, 128), f32)).astype(bf16)

    def ptile(w):  # [D, GD] -> [128, KT, GD] partition-contiguous
        return np.ascontiguousarray(
            w.reshape(KT, 128, GD).transpose(1, 0, 2))

    in_maps = []
    for c in range(N_CORES):
        b, g = divmod(c, HPC)
        sl = slice(GD * g, GD * (g + 1))
        xTb = np.ascontiguousarray(x[b].T)
        # [D, T] -> [128, 4, KT, 512] chunk-major fp8
        x8b = np.ascontiguousarray(
            xTb.astype(fp8).reshape(KT, 128, 4, 512).transpose(1, 2, 0, 3))
        in_maps.append({
            "xT": xTb.astype(bf16),
            "x8": x8b,
            "wqT": ptile((WS * Wq[sl, :].T).astype(fp8)),
            "wkT": ptile((WS * Wk[sl, :].T).astype(fp8)),
            "wvT": ptile(Wv[sl, :].T.astype(bf16)),
            "wpT": np.ascontiguousarray(Wp[:, sl].T, dtype=f32),
            "bq2": np.ascontiguousarray(
                WS * bq[sl].reshape(2, 128).T, dtype=f32),
            "bk2": np.ascontiguousarray(
                WS * bk[sl].reshape(2, 128).T, dtype=f32),
            "bvb": np.broadcast_to(bv[sl], (128, GD)).astype(f32),
            "mask01": mask01,
        })
    return in_maps


def run(inputs, trace=False):
    """Run the SPMD kernel; returns (output [B,T,D] f32, BassKernelResults)."""
    if "nc" not in _cache:
        _cache["nc"] = _build()
    nc = _cache["nc"]
    in_maps = _shard(**inputs)
    if trace:
        _install_ntff_hook()
    res = bass_utils.run_bass_kernel_spmd(
        nc, in_maps, core_ids=list(range(N_CORES)), trace=trace,
    )
    bp = np.asarray(inputs["bp"], dtype=np.float32)
    out = np.empty((B, T, D), dtype=np.float32)
    for b in range(B):
        acc = res.results[4 * b]["out"].astype(np.float32)
        for g in range(1, HPC):
            acc = acc + res.results[4 * b + g]["out"]
        out[b] = acc + bp
    return out, res


def kernel(**inputs):
    out, _ = run(inputs, trace=False)
    return out


def _install_ntff_hook():
    """antenv.axon_hooks is absent on this image; inject it so
    run_bass_kernel_spmd(trace=True) can capture NTFF profiles."""
    import sys, types
    if "antenv.axon_hooks" in sys.modules:
        return
    try:
        mod = types.ModuleType("antenv.axon_hooks")
        mod._hook = None
        mod.set_axon_ntff_profile_hook = lambda h: setattr(mod, "_hook", h)
        mod.get_axon_ntff_profile_hook = lambda: mod._hook
        sys.modules["antenv.axon_hooks"] = mod
        import antenv
        antenv.axon_hooks = mod
        from trn_agent_boot.trn_boot import _ntff_profile_via_ctypes
        mod.set_axon_ntff_profile_hook(
            _ntff_profile_via_ctypes("/opt/axon/libaxon_pjrt.so"))
    except Exception:
        pass


# revision 49
# speedup vs baseline: 1.1410x; 1.0945x over previous
"""Causal self-attention (B=2, T=2048, D=1024, H=16, Dh=64) on 8 TRN2 cores.

Sharding: core c = 4*b + g -> batch b (data parallel), head group g of 4
heads (tensor parallel on heads for Wq/Wk/Wv, column-split of the proj
input with the resulting partial-sum reduction done host-side at unshard).

Per-core dataflow (layouts chosen so no on-device transposes are needed):
  qT,kT [256, 2048] bf16 = W{q,k}_g @ x.T  (fp8 DoubleRow projections; x8
  is shipped pre-cast fp8 from the host so no on-device casts are needed.
  Note: on HW, DR costs 1 cy/output-col like bf16 -- its win is the 2x
  contraction depth per instruction, NOT a faster column rate.)
  v     [t-block 128, 4 heads x (64 v | 64 ones)] bf16
  attention, transposed: PT[tk, tq] = kT_h.T @ qT_h (bf16), exp on
  ACT -> bf16, causal mask as post-exp 0/1 multiply on GPSIMD,
  AV: yT[d, tq] + softmax column sums free via the ones columns of v
  normalize: yT * recip(sums) -> ytsb [256, 2048] f32r (proj lhsT layout)
  proj partial: out[t, :] = ytsb.T-block @ Wp_gT  (f32r)
Host: out[b] = sum_g partial[4b+g] + bp.

Segment order runs all Ti=0 (first T-half) segs for the 4 heads first, so
the first half of the output projection (and its out-DMA traffic) spreads
over the Ti=1 segs instead of bunching at the tail.
"""

import numpy as np

import concourse.bass as bass
import concourse.mybir as mybir
import concourse.tile as tile
from concourse import bacc
from concourse import bass_utils

F32 = mybir.dt.float32
F32R = mybir.dt.float32r
BF16 = mybir.dt.bfloat16
FP8 = mybir.dt.float8e4
DR = mybir.MatmulPerfMode.DoubleRow
WS = 16.0          # host scale on Wq/Wk (and bq/bk) so fp8 keeps mantissa

B, T, D = 2, 2048, 1024
H, DH = 16, 64
N_CORES = 8
HPC = 4            # heads per core
GD = HPC * DH      # 256 feature cols per core
KT = D // 128      # 8 k-tiles over the model dim
TB = T // 128      # 16 t-blocks of 128
SCL = 0.125        # logit scale 1/sqrt(Dh)

_cache = {}


def _build():
    nc = bacc.Bacc("TRN2", target_bir_lowering=False, debug=False,
                   num_devices=N_CORES)

    xT_d = nc.dram_tensor("xT", [D, T], BF16, kind="ExternalInput")
    # x8: host-cast fp8 copy of x, chunk-major [128, 4, KT, 512] so each
    # 512-col chunk is one contiguous-per-partition DMA
    x8_d = nc.dram_tensor("x8", [128, 4, KT, 512], FP8, kind="ExternalInput")
    wqT_d = nc.dram_tensor("wqT", [128, KT, GD], FP8, kind="ExternalInput")
    wkT_d = nc.dram_tensor("wkT", [128, KT, GD], FP8, kind="ExternalInput")
    wvT_d = nc.dram_tensor("wvT", [128, KT, GD], BF16, kind="ExternalInput")
    wpT_d = nc.dram_tensor("wpT", [GD, D], F32R, kind="ExternalInput")
    bq_d = nc.dram_tensor("bq2", [128, 2], F32, kind="ExternalInput")
    bk_d = nc.dram_tensor("bk2", [128, 2], F32, kind="ExternalInput")
    bvb_d = nc.dram_tensor("bvb", [128, GD], F32, kind="ExternalInput")
    msk_d = nc.dram_tensor("mask01", [128, 128], BF16, kind="ExternalInput")
    out_d = nc.dram_tensor("out", [T, D], BF16, kind="ExternalOutput")
    wrm_d = nc.dram_tensor("wrm", [128, 1], F32, kind="ExternalOutput")

    with tile.TileContext(nc) as tc:
        with (
            tc.tile_pool(name="const", bufs=1) as cp,
            tc.tile_pool(name="big", bufs=1) as bp_,
            tc.tile_pool(name="work", bufs=4) as wp_,
            tc.tile_pool(name="outp", bufs=6) as op_,
            tc.tile_pool(name="pA", bufs=2, space="PSUM") as pA,
            tc.tile_pool(name="pB", bufs=1, space="PSUM") as pB,
            tc.tile_pool(name="pC", bufs=2, space="PSUM") as pC,
        ):
            # ---- loads, ordered by need-time. First matmul needs wq + x8
            # chunk 0 only; queue issue cost (~0.6us each) is the real head
            # constraint, so big tensors ride single DMAs. ----
            wq = cp.tile([128, KT, GD], FP8, tag="wq", name="wq")
            wk = cp.tile([128, KT, GD], FP8, tag="wk", name="wk")
            wv = cp.tile([128, KT, GD], BF16, tag="wv", name="wv")
            bq2 = cp.tile([128, 2], F32, tag="bq2", name="bq2")
            bk2 = cp.tile([128, 2], F32, tag="bk2", name="bk2")
            bvb = cp.tile([128, GD], F32, tag="bvb", name="bvb")
            msk = cp.tile([128, 128], BF16, tag="msk", name="msk")
            x8 = cp.tile([128, 4, KT, 512], FP8, tag="x8", name="x8")
            xt = [cp.tile([128, T], BF16, tag=f"xt{k}", name=f"xt{k}")
                  for k in range(KT)]

            # gpsimd issues no loads: its queue must stay clear for the
            # exp->mask->AV critical chain
            # first matmul needs only wq k-tiles 0:2 + x8 chunk-0 k-tiles
            # 0:2; split those loads so transfer time off the critical path
            nc.scalar.dma_start(wq[:, 0:2], wqT_d[:, 0:2])
            nc.sync.dma_start(x8[:, 0, 0:2], x8_d[:, 0, 0:2])
            nc.scalar.dma_start(wq[:, 2:4], wqT_d[:, 2:4])
            nc.sync.dma_start(x8[:, 0, 2:4], x8_d[:, 0, 2:4])
            nc.scalar.dma_start(wq[:, 4:KT], wqT_d[:, 4:KT])
            nc.sync.dma_start(x8[:, 0, 4:KT], x8_d[:, 0, 4:KT])
            nc.scalar.dma_start(bq2[:], bq_d[:])
            # dummy exp during the head: pulls the ACT table load (~1.3us)
            # off the first attention block's critical path
            wrmup = cp.tile([128, 2], F32, tag="wrmup", name="wrmup")
            nc.scalar.activation(wrmup[:], bq2[:],
                                 mybir.ActivationFunctionType.Exp,
                                 scale=1.0)
            nc.sync.dma_start(wk[:], wkT_d[:])
            nc.scalar.dma_start(bk2[:], bk_d[:])
            nc.sync.dma_start(x8[:, 1, 0:4], x8_d[:, 1, 0:4])
            nc.sync.dma_start(x8[:, 1, 4:KT], x8_d[:, 1, 4:KT])
            nc.scalar.dma_start(x8[:, 2], x8_d[:, 2])
            nc.scalar.dma_start(x8[:, 3], x8_d[:, 3])
            nc.sync.dma_start(msk[:], msk_d[:])
            nc.sync.dma_start(wv[:], wvT_d[:])
            nc.sync.dma_start(bvb[:], bvb_d[:])
            # xt rows: needed by v_groups (first ~12us in); one DMA per row
            for k in range(KT):
                (nc.sync if k % 2 == 0 else nc.scalar).dma_start(
                    xt[k][:], xT_d[k * 128:(k + 1) * 128, :])
            wpt = []
            for p in range(2):
                t_ = cp.tile([128, D], F32R, tag=f"wp{p}", name=f"wp{p}")
                nc.scalar.dma_start(t_[:], wpT_d[p * 128:(p + 1) * 128, :])
                wpt.append(t_)

            qt = [bp_.tile([128, T], BF16, tag=f"qt{m}", name=f"qt{m}")
                  for m in range(2)]
            kt = [bp_.tile([128, T], BF16, tag=f"kt{m}", name=f"kt{m}")
                  for m in range(2)]
            ytsb = [bp_.tile([128, T], F32R, tag=f"yt{p}", name=f"yt{p}")
                    for p in range(2)]
            vt = [bp_.tile([128, 4, 2, DH], BF16, tag=f"v{t}", name=f"v{t}")
                  for t in range(TB)]

            def qk_group(dst, w, b2, m, n):
                # fp8 DoubleRow: two k-tiles per matmul (lhsT [128,2,128],
                # rhs [128,2,512]) -- on HW, DR costs 1 cy per output col
                # like bf16, but packs 2x contraction depth per instruction,
                # so 4 full-width matmuls cover all 8 k-tiles.
                # Values carry a x16 host scale; exp scale compensates.
                ps = pC.tile([128, 512], F32, tag=pC.name, name="psqk")
                for kp in range(4):
                    nc.tensor.matmul(
                        ps[:],
                        w[:, 2 * kp:2 * kp + 2, m * 128:(m + 1) * 128],
                        x8[:, n, 2 * kp:2 * kp + 2, :],
                        start=(kp == 0), stop=(kp == 3),
                        perf_mode=DR,
                    )
                nc.vector.tensor_scalar_add(
                    dst[m][:, n * 512:(n + 1) * 512], ps[:], b2[:, m:m + 1],
                )

            def v_group(t):
                ps = pC.tile([128, 512], F32, tag=pC.name, name="psv")
                for k in range(KT):
                    nc.tensor.matmul(
                        ps[:, 0:GD],
                        xt[k][:, t * 128:(t + 1) * 128],
                        wv[:, k, :],
                        start=(k == 0), stop=(k == KT - 1),
                    )
                nc.vector.tensor_add(
                    vt[t][:, :, 0, :],
                    ps[:, 0:GD].rearrange("p (h d) -> p h d", h=4),
                    bvb.rearrange("p (h d) -> p h d", h=4),
                )

            def proj_group(t, copy_eng):
                # each 512-col half is copied then DMA'd out immediately so
                # the final-output tail pipelines at half-tile granularity.
                # Late groups (copy_eng == "act") run during the tail: the
                # PSUM->SBUF copies go to the then-idle scalar engine, and
                # the out-DMAs split across queues so the last transfers
                # ride several DMA engines instead of one.
                ob = op_.tile([128, 1024], BF16, tag="ob", name="ob")
                for n in range(2):
                    po = pC.tile([128, 512], F32, tag=pC.name, name="pso")
                    for p in range(2):
                        nc.tensor.matmul(
                            po[:],
                            ytsb[p][:, 128 * t:128 * (t + 1)],
                            wpt[p][:, 512 * n:512 * (n + 1)],
                            start=(p == 0), stop=(p == 1),
                        )
                    if copy_eng == "act":
                        nc.scalar.copy(ob[:, 512 * n:512 * (n + 1)], po[:])
                        for h_ in range(2):
                            cs = slice(512 * n + 256 * h_,
                                       512 * n + 256 * (h_ + 1))
                            eng = (nc.sync, nc.scalar,
                                   nc.gpsimd)[(2 * t + 2 * n + h_) % 3]
                            eng.dma_start(
                                out_d[128 * t:128 * (t + 1), cs], ob[:, cs])
                    else:
                        nc.vector.tensor_copy(
                            ob[:, 512 * n:512 * (n + 1)], po[:])
                        nc.sync.dma_start(
                            out_d[128 * t:128 * (t + 1),
                                  512 * n:512 * (n + 1)],
                            ob[:, 512 * n:512 * (n + 1)])

            def attention_seg(Ti, h, fillers, every, last=False,
                              post=None):
                hp, j = h // 2, h % 2
                ytp = pB.tile([128, 1024], F32, tag=pB.name, name="psyt")
                nblk = 8 * (Ti + 1)
                SKEW = 3       # AV trails QK/exp: the PE never
                pend = []      # waits on an exp that was just issued
                def do_av(tkb, ptsb):
                    s = max(0, 128 * tkb - 1024 * Ti)
                    for bk in range(2):
                        c0, c1 = max(s, 512 * bk), 512 * (bk + 1)
                        if c0 >= c1:
                            continue
                        nc.tensor.matmul(
                            ytp[:, c0:c1],
                            vt[tkb][:, h, :, :].rearrange("p a d -> p (a d)"),
                            ptsb[:, c0:c1],
                            start=(tkb == 0), stop=(tkb == nblk - 1),
                        )
                for tkb in range(nblk + SKEW):
                    if tkb < nblk:
                        s = max(0, 128 * tkb - 1024 * Ti)
                        pt = pA.tile([128, 1024], F32, tag=pA.name,
                                     name="pspt")
                        for bk in range(2):
                            c0, c1 = max(s, 512 * bk), 512 * (bk + 1)
                            if c0 >= c1:
                                continue
                            nc.tensor.matmul(
                                pt[:, c0:c1],
                                kt[hp][64 * j:64 * j + 64,
                                       128 * tkb:128 * (tkb + 1)],
                                qt[hp][64 * j:64 * j + 64,
                                       1024 * Ti + c0:1024 * Ti + c1],
                                start=True, stop=True,
                            )
                        ptsb = wp_.tile([128, 1024], BF16, tag="ptsb",
                                        name="ptsb", bufs=6)
                        nc.scalar.activation(
                            ptsb[:, s:1024], pt[:, s:1024],
                            mybir.ActivationFunctionType.Exp,
                            scale=SCL / (WS * WS),
                        )
                        if 128 * tkb >= 1024 * Ti:  # diagonal block
                            nc.gpsimd.tensor_mul(
                                ptsb[:, s:s + 128], ptsb[:, s:s + 128],
                                msk[:],
                            )
                        pend.append((tkb, ptsb))
                    if tkb >= SKEW:
                        do_av(*pend.pop(0))
                    if fillers and tkb % every == every - 1:
                        fillers.pop(0)()
                while pend:
                    do_av(*pend.pop(0))
                # free the PSUM accumulator promptly; 1/sums via the custom
                # fast-approx DVE reciprocal (~5x cheaper than InstReciprocal,
                # 18 good bits -- plenty for softmax normalization)
                if not last:
                    src = wp_.tile([128, 1024], F32, tag="ysb", name="ysb")
                    nc.vector.tensor_copy(src[:], ytp[:])
                else:
                    src = ytp
                # custom-DVE fast reciprocal: in/out must share a partition
                # base (offset-crossing APs feed it garbage -> NaN), and
                # tensor_tensor needs equal input bases -- so recip at base
                # 64, then a single-input gpsimd copy crosses down to base 0
                rc = wp_.tile([128, 1024], F32, tag="recip", name="recip")
                cp_eng = nc.gpsimd if last else nc.vector
                for q in range(4):
                    cs = slice(256 * q, 256 * (q + 1))
                    nc.vector.reciprocal_approx_fast(rc[:, cs], src[:, cs])
                    cp_eng.tensor_copy(rc[0:64, cs], rc[64:128, cs])
                    nc.vector.tensor_mul(
                        ytsb[hp][64 * j:64 * j + 64,
                                 1024 * Ti + 256 * q:1024 * Ti + 256 * (q + 1)],
                        src[0:64, cs], rc[0:64, cs],
                    )
                    if post:
                        post.pop(0)()
                        post.pop(0)()

            # ---- schedule ----
            # all ones-column memsets up front: a memset inside v_group
            # lands on the gpsimd queue behind mask-multiplies that wait on
            # exp, chaining the v fillers (and the PE) to the exp pace
            for t in range(TB):
                nc.gpsimd.memset(vt[t][:, :, 1, :], 1.0)
            # head clock pre-ramp: the PE idles ~10us while loads land, then
            # pays ~3.5us of half-clock p-state ramp on its first real
            # matmuls. Dummy matmuls on the memset-only vt[0] (ready ~1us,
            # no DMA dependency) are interleaved BETWEEN the projection
            # groups: if loads are late they fill the stalls and hold the
            # clock; if loads are early each group is delayed by at most
            # two dummies (~0.4us). All accumulate into one pB tile (free
            # until seg 0) so the pC rotation is untouched; a reduce_max
            # into a debug output prevents elimination.
            wrm = bp_.tile([128, 1], F32, tag="wrm", name="wrm")
            wps = pB.tile([128, 512], F32, tag=pB.name, name="pswrm")
            wct = [0]

            def warm(k, stop=False):
                for i in range(k):
                    nc.tensor.matmul(
                        wps[:],
                        vt[0][:, 0, :, :].rearrange("p a d -> p (a d)"),
                        vt[0].rearrange("p h a d -> p (h a d)"),
                        start=(wct[0] == 0),
                        stop=(stop and i == k - 1),
                    )
                    wct[0] += 1

            # q/k projections for the first T-half (n=0,1) of BOTH m-chunks
            # up front: every Ti=0 seg only touches q/k columns 0:1024.
            # The three head-start v groups (seg 0's SKEW lead) sit between
            # the n=0 and n=1 groups to cover the x8 chunk-1 transfer.
            warm(8)
            for m in range(2):
                qk_group(qt, wq, bq2, m, 0)
                warm(2)
                qk_group(kt, wk, bk2, m, 0)
                warm(2)
            for t in range(3):
                v_group(t)
                warm(1)
            for m in range(2):
                qk_group(qt, wq, bq2, m, 1)
                warm(2)
                qk_group(kt, wk, bk2, m, 1)
                warm(2 if m == 0 else 0)
            warm(1, stop=True)
            nc.vector.reduce_max(wrm[:], wps[:], axis=mybir.AxisListType.X)

            def f_qk(n):  # remaining projection column chunks
                return [lambda m=m, w=w, b=b, d=d: qk_group(d, w, b, m, n)
                        for m in range(2) for (d, w, b) in
                        [(qt, wq, bq2), (kt, wk, bk2)]]
            f_v03 = [lambda t=t: v_group(t) for t in range(3, 8)]
            f_v = [lambda t=t: v_group(t) for t in range(8, 16)]
            f_p0 = [lambda t=t: proj_group(t, "vec") for t in range(8)]

            n2 = f_qk(2)
            # Ti=0 for all heads first, then Ti=1; the first output half's
            # proj groups drip through the Ti=1 segs
            attention_seg(0, 0, [n2[0], f_v03[0], n2[1], f_v03[1], n2[2],
                                 f_v03[2], n2[3], f_v03[3], f_v03[4]], 1)
            attention_seg(0, 1, f_qk(3), 2)
            attention_seg(0, 2, f_v[0:4], 2)
            attention_seg(0, 3, f_v[4:8], 2)
            attention_seg(1, 0, f_p0[0:3], 4)
            attention_seg(1, 1, f_p0[3:6], 4)
            attention_seg(1, 2, f_p0[6:8], 8)
            # proj for the second T-half drips into the last seg's normalize
            # quarters (each 256-col quarter unlocks two 128-row t-blocks)
            f_p1 = [lambda t=t: proj_group(t, "act") for t in range(8, 16)]
            attention_seg(1, 3, [], 4, last=True, post=f_p1)
            nc.sync.dma_start(wrm_d[:], wrm[:])

    nc.compile()
    return nc


def _shard(x, Wq, bq, Wk, bk, Wv, bv, Wp, bp):
    import ml_dtypes
    f32 = np.float32
    bf16 = ml_dtypes.bfloat16
    fp8 = ml_dtypes.float8_e4m3fn
    mask01 = np.triu(np.ones((128, 128), f32)).astype(bf16)

    def ptile(w):  # [D, GD] -> [128, KT, GD] partition-contiguous
        return np.ascontiguousarray(
            w.reshape(KT, 128, GD).transpose(1, 0, 2))

    in_maps = []
    for c in range(N_CORES):
        b, g = divmod(c, HPC)
        sl = slice(GD * g, GD * (g + 1))
        xTb = np.ascontiguousarray(x[b].T)
        # [D, T] -> [128, 4, KT, 512] chunk-major fp8
        x8b = np.ascontiguousarray(
            xTb.astype(fp8).reshape(KT, 128, 4, 512).transpose(1, 2, 0, 3))
        in_maps.append({
            "xT": xTb.astype(bf16),
            "x8": x8b,
            "wqT": ptile((WS * Wq[sl, :].T).astype(fp8)),
            "wkT": ptile((WS * Wk[sl, :].T).astype(fp8)),
            "wvT": ptile(Wv[sl, :].T.astype(bf16)),
            "wpT": np.ascontiguousarray(Wp[:, sl].T, dtype=f32),
            "bq2": np.ascontiguousarray(
                WS * bq[sl].reshape(2, 128).T, dtype=f32),
            "bk2": np.ascontiguousarray(
                WS * bk[sl].reshape(2, 128).T, dtype=f32),
            "bvb": np.broadcast_to(bv[sl], (128, GD)).astype(f32),
            "mask01": mask01,
        })
    return in_maps


def run(inputs, trace=False):
    """Run the SPMD kernel; returns (output [B,T,D] f32, BassKernelResults)."""
    if "nc" not in _cache:
        _cache["nc"] = _build()
    nc = _cache["nc"]
    in_maps = _shard(**inputs)
    if trace:
        _install_ntff_hook()
    res = bass_utils.run_bass_kernel_spmd(
        nc, in_maps, core_ids=list(range(N_CORES)), trace=trace,
    )
    bp = np.asarray(inputs["bp"], dtype=np.float32)
    out = np.empty((B, T, D), dtype=np.float32)
    for b in range(B):
        acc = res.results[4 * b]["out"].astype(np.float32)
        for g in range(1, HPC):
            acc = acc + res.results[4 * b + g]["out"]
        out[b] = acc + bp
    return out, res


def kernel(**inputs):
    out, _ = run(inputs, trace=False)
    return out


def _install_ntff_hook():
    """antenv.axon_hooks is absent on this image; inject it so
    run_bass_kernel_spmd(trace=True) can capture NTFF profiles."""
    import sys, types
    if "antenv.axon_hooks" in sys.modules:
        return
    try:
        mod = types.ModuleType("antenv.axon_hooks")
        mod._hook = None
        mod.set_axon_ntff_profile_hook = lambda h: setattr(mod, "_hook", h)
        mod.get_axon_ntff_profile_hook = lambda: mod._hook
        sys.modules["antenv.axon_hooks"] = mod
        import antenv
        antenv.axon_hooks = mod
        from trn_agent_boot.trn_boot import _ntff_profile_via_ctypes
        mod.set_axon_ntff_profile_hook(
            _ntff_profile_via_ctypes("/opt/axon/libaxon_pjrt.so"))
    except Exception:
        pass


# revision 50
# speedup vs baseline: 1.1890x; 1.0420x over previous
"""Causal self-attention (B=2, T=2048, D=1024, H=16, Dh=64) on 8 TRN2 cores.

Sharding: core c = 4*b + g -> batch b (data parallel), head group g of 4
heads (tensor parallel on heads for Wq/Wk/Wv, column-split of the proj
input with the resulting partial-sum reduction done host-side at unshard).

Per-core dataflow (layouts chosen so no on-device transposes are needed):
  qT,kT [256, 2048] bf16 = W{q,k}_g @ x.T  (fp8 DoubleRow projections; x8
  is shipped pre-cast fp8 from the host so no on-device casts are needed.
  Note: on HW, DR costs 1 cy/output-col like bf16 -- its win is the 2x
  contraction depth per instruction, NOT a faster column rate.)
  v     [t-block 128, 4 heads x (64 v | 64 ones)] bf16
  attention, transposed: PT[tk, tq] = kT_h.T @ qT_h (bf16), exp on
  ACT -> bf16, causal mask as post-exp 0/1 multiply on GPSIMD,
  AV: yT[d, tq] + softmax column sums free via the ones columns of v
  normalize: yT * recip(sums) -> ytsb [256, 2048] f32r (proj lhsT layout)
  proj partial: out[t, :] = ytsb.T-block @ Wp_gT  (f32r)
Host: out[b] = sum_g partial[4b+g] + bp.

Segment order runs all Ti=0 (first T-half) segs for the 4 heads first, so
the first half of the output projection (and its out-DMA traffic) spreads
over the Ti=1 segs instead of bunching at the tail.
"""

import numpy as np

import concourse.bass as bass
import concourse.mybir as mybir
import concourse.tile as tile
from concourse import bacc
from concourse import bass_utils

F32 = mybir.dt.float32
F32R = mybir.dt.float32r
BF16 = mybir.dt.bfloat16
FP8 = mybir.dt.float8e4
DR = mybir.MatmulPerfMode.DoubleRow
WS = 16.0          # host scale on Wq/Wk (and bq/bk) so fp8 keeps mantissa

B, T, D = 2, 2048, 1024
H, DH = 16, 64
N_CORES = 8
HPC = 4            # heads per core
GD = HPC * DH      # 256 feature cols per core
KT = D // 128      # 8 k-tiles over the model dim
TB = T // 128      # 16 t-blocks of 128
SCL = 0.125        # logit scale 1/sqrt(Dh)

_cache = {}


def _build():
    nc = bacc.Bacc("TRN2", target_bir_lowering=False, debug=False,
                   num_devices=N_CORES)

    xT_d = nc.dram_tensor("xT", [D, T], BF16, kind="ExternalInput")
    # x8: host-cast fp8 copy of x, chunk-major [128, 4, KT, 512] so each
    # 512-col chunk is one contiguous-per-partition DMA
    x8_d = nc.dram_tensor("x8", [128, 4, KT, 512], FP8, kind="ExternalInput")
    wqT_d = nc.dram_tensor("wqT", [128, KT, GD], FP8, kind="ExternalInput")
    wkT_d = nc.dram_tensor("wkT", [128, KT, GD], FP8, kind="ExternalInput")
    wvT_d = nc.dram_tensor("wvT", [128, KT, GD], BF16, kind="ExternalInput")
    wpT_d = nc.dram_tensor("wpT", [GD, D], F32R, kind="ExternalInput")
    bq_d = nc.dram_tensor("bq2", [128, 2], F32, kind="ExternalInput")
    bk_d = nc.dram_tensor("bk2", [128, 2], F32, kind="ExternalInput")
    bvb_d = nc.dram_tensor("bvb", [128, GD], F32, kind="ExternalInput")
    msk_d = nc.dram_tensor("mask01", [128, 128], BF16, kind="ExternalInput")
    out_d = nc.dram_tensor("out", [T, D], BF16, kind="ExternalOutput")
    wrm_d = nc.dram_tensor("wrm", [128, 1], F32, kind="ExternalOutput")

    with tile.TileContext(nc) as tc:
        with (
            tc.tile_pool(name="const", bufs=1) as cp,
            tc.tile_pool(name="big", bufs=1) as bp_,
            tc.tile_pool(name="work", bufs=4) as wp_,
            tc.tile_pool(name="outp", bufs=6) as op_,
            tc.tile_pool(name="pA", bufs=2, space="PSUM") as pA,
            tc.tile_pool(name="pB", bufs=1, space="PSUM") as pB,
            tc.tile_pool(name="pC", bufs=2, space="PSUM") as pC,
        ):
            # ---- loads, ordered by need-time. First matmul needs wq + x8
            # chunk 0 only; queue issue cost (~0.6us each) is the real head
            # constraint, so big tensors ride single DMAs. ----
            wq = cp.tile([128, KT, GD], FP8, tag="wq", name="wq")
            wk = cp.tile([128, KT, GD], FP8, tag="wk", name="wk")
            wv = cp.tile([128, KT, GD], BF16, tag="wv", name="wv")
            bq2 = cp.tile([128, 2], F32, tag="bq2", name="bq2")
            bk2 = cp.tile([128, 2], F32, tag="bk2", name="bk2")
            bvb = cp.tile([128, GD], F32, tag="bvb", name="bvb")
            msk = cp.tile([128, 128], BF16, tag="msk", name="msk")
            x8 = cp.tile([128, 4, KT, 512], FP8, tag="x8", name="x8")
            xt = [cp.tile([128, T], BF16, tag=f"xt{k}", name=f"xt{k}")
                  for k in range(KT)]

            # gpsimd issues no loads: its queue must stay clear for the
            # exp->mask->AV critical chain
            # first matmul needs only wq k-tiles 0:2 + x8 chunk-0 k-tiles
            # 0:2; split those loads so transfer time off the critical path
            nc.scalar.dma_start(wq[:, 0:2], wqT_d[:, 0:2])
            nc.sync.dma_start(x8[:, 0, 0:2], x8_d[:, 0, 0:2])
            nc.scalar.dma_start(wq[:, 2:4], wqT_d[:, 2:4])
            nc.sync.dma_start(x8[:, 0, 2:4], x8_d[:, 0, 2:4])
            nc.scalar.dma_start(wq[:, 4:KT], wqT_d[:, 4:KT])
            nc.sync.dma_start(x8[:, 0, 4:KT], x8_d[:, 0, 4:KT])
            nc.scalar.dma_start(bq2[:], bq_d[:])
            # dummy exp during the head: pulls the ACT table load (~1.3us)
            # off the first attention block's critical path
            wrmup = cp.tile([128, 2], F32, tag="wrmup", name="wrmup")
            nc.scalar.activation(wrmup[:], bq2[:],
                                 mybir.ActivationFunctionType.Exp,
                                 scale=1.0)
            nc.sync.dma_start(wk[:], wkT_d[:])
            nc.scalar.dma_start(bk2[:], bk_d[:])
            # everything below rides the sync ring: the scalar queue must
            # drain its issues before exp(0) can dispatch, and each leftover
            # load there becomes a ~0.65us bubble between attention exps
            nc.sync.dma_start(x8[:, 1, 0:4], x8_d[:, 1, 0:4])
            nc.sync.dma_start(x8[:, 1, 4:KT], x8_d[:, 1, 4:KT])
            nc.sync.dma_start(x8[:, 2], x8_d[:, 2])
            nc.sync.dma_start(x8[:, 3], x8_d[:, 3])
            nc.sync.dma_start(msk[:], msk_d[:])
            nc.sync.dma_start(wv[:], wvT_d[:])
            nc.sync.dma_start(bvb[:], bvb_d[:])
            # xt rows: needed by v_groups (first ~12us in); one DMA per row
            for k in range(KT):
                nc.sync.dma_start(xt[k][:], xT_d[k * 128:(k + 1) * 128, :])
            wpt = []
            for p in range(2):
                t_ = cp.tile([128, D], F32R, tag=f"wp{p}", name=f"wp{p}")
                nc.sync.dma_start(t_[:], wpT_d[p * 128:(p + 1) * 128, :])
                wpt.append(t_)

            qt = [bp_.tile([128, T], BF16, tag=f"qt{m}", name=f"qt{m}")
                  for m in range(2)]
            kt = [bp_.tile([128, T], BF16, tag=f"kt{m}", name=f"kt{m}")
                  for m in range(2)]
            ytsb = [bp_.tile([128, T], F32R, tag=f"yt{p}", name=f"yt{p}")
                    for p in range(2)]
            vt = [bp_.tile([128, 4, 2, DH], BF16, tag=f"v{t}", name=f"v{t}")
                  for t in range(TB)]

            def qk_group(dst, w, b2, m, n):
                # fp8 DoubleRow: two k-tiles per matmul (lhsT [128,2,128],
                # rhs [128,2,512]) -- on HW, DR costs 1 cy per output col
                # like bf16, but packs 2x contraction depth per instruction,
                # so 4 full-width matmuls cover all 8 k-tiles.
                # Values carry a x16 host scale; exp scale compensates.
                ps = pC.tile([128, 512], F32, tag=pC.name, name="psqk")
                for kp in range(4):
                    nc.tensor.matmul(
                        ps[:],
                        w[:, 2 * kp:2 * kp + 2, m * 128:(m + 1) * 128],
                        x8[:, n, 2 * kp:2 * kp + 2, :],
                        start=(kp == 0), stop=(kp == 3),
                        perf_mode=DR,
                    )
                nc.vector.tensor_scalar_add(
                    dst[m][:, n * 512:(n + 1) * 512], ps[:], b2[:, m:m + 1],
                )

            def v_group(t):
                ps = pC.tile([128, 512], F32, tag=pC.name, name="psv")
                for k in range(KT):
                    nc.tensor.matmul(
                        ps[:, 0:GD],
                        xt[k][:, t * 128:(t + 1) * 128],
                        wv[:, k, :],
                        start=(k == 0), stop=(k == KT - 1),
                    )
                nc.vector.tensor_add(
                    vt[t][:, :, 0, :],
                    ps[:, 0:GD].rearrange("p (h d) -> p h d", h=4),
                    bvb.rearrange("p (h d) -> p h d", h=4),
                )

            def proj_group(t, copy_eng):
                # each 512-col half is copied then DMA'd out immediately so
                # the final-output tail pipelines at half-tile granularity.
                # Late groups (copy_eng == "act") run during the tail: the
                # PSUM->SBUF copies go to the then-idle scalar engine, and
                # the out-DMAs split across queues so the last transfers
                # ride several DMA engines instead of one.
                ob = op_.tile([128, 1024], BF16, tag="ob", name="ob")
                for n in range(2):
                    po = pC.tile([128, 512], F32, tag=pC.name, name="pso")
                    for p in range(2):
                        nc.tensor.matmul(
                            po[:],
                            ytsb[p][:, 128 * t:128 * (t + 1)],
                            wpt[p][:, 512 * n:512 * (n + 1)],
                            start=(p == 0), stop=(p == 1),
                        )
                    if copy_eng == "act":
                        nc.scalar.copy(ob[:, 512 * n:512 * (n + 1)], po[:])
                        for h_ in range(2):
                            cs = slice(512 * n + 256 * h_,
                                       512 * n + 256 * (h_ + 1))
                            eng = (nc.sync, nc.scalar,
                                   nc.gpsimd)[(2 * t + 2 * n + h_) % 3]
                            eng.dma_start(
                                out_d[128 * t:128 * (t + 1), cs], ob[:, cs])
                    else:
                        nc.vector.tensor_copy(
                            ob[:, 512 * n:512 * (n + 1)], po[:])
                        nc.sync.dma_start(
                            out_d[128 * t:128 * (t + 1),
                                  512 * n:512 * (n + 1)],
                            ob[:, 512 * n:512 * (n + 1)])

            def attention_seg(Ti, h, fillers, every, last=False,
                              post=None):
                hp, j = h // 2, h % 2
                ytp = pB.tile([128, 1024], F32, tag=pB.name, name="psyt")
                nblk = 8 * (Ti + 1)
                SKEW = 3       # AV trails QK/exp: the PE never
                pend = []      # waits on an exp that was just issued
                def do_av(tkb, ptsb):
                    s = max(0, 128 * tkb - 1024 * Ti)
                    for bk in range(2):
                        c0, c1 = max(s, 512 * bk), 512 * (bk + 1)
                        if c0 >= c1:
                            continue
                        nc.tensor.matmul(
                            ytp[:, c0:c1],
                            vt[tkb][:, h, :, :].rearrange("p a d -> p (a d)"),
                            ptsb[:, c0:c1],
                            start=(tkb == 0), stop=(tkb == nblk - 1),
                        )
                for tkb in range(nblk + SKEW):
                    if tkb < nblk:
                        s = max(0, 128 * tkb - 1024 * Ti)
                        pt = pA.tile([128, 1024], F32, tag=pA.name,
                                     name="pspt")
                        for bk in range(2):
                            c0, c1 = max(s, 512 * bk), 512 * (bk + 1)
                            if c0 >= c1:
                                continue
                            nc.tensor.matmul(
                                pt[:, c0:c1],
                                kt[hp][64 * j:64 * j + 64,
                                       128 * tkb:128 * (tkb + 1)],
                                qt[hp][64 * j:64 * j + 64,
                                       1024 * Ti + c0:1024 * Ti + c1],
                                start=True, stop=True,
                            )
                        ptsb = wp_.tile([128, 1024], BF16, tag="ptsb",
                                        name="ptsb", bufs=6)
                        nc.scalar.activation(
                            ptsb[:, s:1024], pt[:, s:1024],
                            mybir.ActivationFunctionType.Exp,
                            scale=SCL / (WS * WS),
                        )
                        if 128 * tkb >= 1024 * Ti:  # diagonal block
                            nc.gpsimd.tensor_mul(
                                ptsb[:, s:s + 128], ptsb[:, s:s + 128],
                                msk[:],
                            )
                        pend.append((tkb, ptsb))
                    if tkb >= SKEW:
                        do_av(*pend.pop(0))
                    if fillers and tkb % every == every - 1:
                        fillers.pop(0)()
                while pend:
                    do_av(*pend.pop(0))
                # free the PSUM accumulator promptly; 1/sums via the custom
                # fast-approx DVE reciprocal (~5x cheaper than InstReciprocal,
                # 18 good bits -- plenty for softmax normalization)
                if not last:
                    src = wp_.tile([128, 1024], F32, tag="ysb", name="ysb")
                    nc.vector.tensor_copy(src[:], ytp[:])
                else:
                    src = ytp
                # custom-DVE fast reciprocal: in/out must share a partition
                # base (offset-crossing APs feed it garbage -> NaN), and
                # tensor_tensor needs equal input bases -- so recip at base
                # 64, then a single-input gpsimd copy crosses down to base 0
                rc = wp_.tile([128, 1024], F32, tag="recip", name="recip")
                cp_eng = nc.gpsimd if last else nc.vector
                for q in range(4):
                    cs = slice(256 * q, 256 * (q + 1))
                    nc.vector.reciprocal_approx_fast(rc[:, cs], src[:, cs])
                    cp_eng.tensor_copy(rc[0:64, cs], rc[64:128, cs])
                    nc.vector.tensor_mul(
                        ytsb[hp][64 * j:64 * j + 64,
                                 1024 * Ti + 256 * q:1024 * Ti + 256 * (q + 1)],
                        src[0:64, cs], rc[0:64, cs],
                    )
                    if post:
                        post.pop(0)()
                        post.pop(0)()

            # ---- schedule ----
            # all ones-column memsets up front: a memset inside v_group
            # lands on the gpsimd queue behind mask-multiplies that wait on
            # exp, chaining the v fillers (and the PE) to the exp pace
            for t in range(TB):
                nc.gpsimd.memset(vt[t][:, :, 1, :], 1.0)
            # head clock pre-ramp: the PE idles ~10us while loads land, then
            # pays ~3.5us of half-clock p-state ramp on its first real
            # matmuls. Dummy matmuls on the memset-only vt[0] (ready ~1us,
            # no DMA dependency) are interleaved BETWEEN the projection
            # groups: if loads are late they fill the stalls and hold the
            # clock; if loads are early each group is delayed by at most
            # two dummies (~0.4us). All accumulate into one pB tile (free
            # until seg 0) so the pC rotation is untouched; a reduce_max
            # into a debug output prevents elimination.
            wrm = bp_.tile([128, 1], F32, tag="wrm", name="wrm")
            wps = pB.tile([128, 512], F32, tag=pB.name, name="pswrm")
            wct = [0]

            def warm(k, stop=False):
                for i in range(k):
                    nc.tensor.matmul(
                        wps[:],
                        vt[0][:, 0, :, :].rearrange("p a d -> p (a d)"),
                        vt[0].rearrange("p h a d -> p (h a d)"),
                        start=(wct[0] == 0),
                        stop=(stop and i == k - 1),
                    )
                    wct[0] += 1

            # q/k projections for the first T-half (n=0,1) of BOTH m-chunks
            # up front: every Ti=0 seg only touches q/k columns 0:1024.
            # The three head-start v groups (seg 0's SKEW lead) sit between
            # the n=0 and n=1 groups to cover the x8 chunk-1 transfer.
            warm(8)
            for m in range(2):
                qk_group(qt, wq, bq2, m, 0)
                warm(2)
                qk_group(kt, wk, bk2, m, 0)
                warm(2)
            for t in range(3):
                v_group(t)
                warm(1)
            for m in range(2):
                qk_group(qt, wq, bq2, m, 1)
                warm(2)
                qk_group(kt, wk, bk2, m, 1)
                warm(2 if m == 0 else 0)
            warm(1, stop=True)
            nc.vector.reduce_max(wrm[:], wps[:], axis=mybir.AxisListType.X)

            def f_qk(n):  # remaining projection column chunks
                return [lambda m=m, w=w, b=b, d=d: qk_group(d, w, b, m, n)
                        for m in range(2) for (d, w, b) in
                        [(qt, wq, bq2), (kt, wk, bk2)]]
            f_v03 = [lambda t=t: v_group(t) for t in range(3, 8)]
            f_v = [lambda t=t: v_group(t) for t in range(8, 16)]
            f_p0 = [lambda t=t: proj_group(t, "vec") for t in range(8)]

            n2 = f_qk(2)
            # Ti=0 for all heads first, then Ti=1; the first output half's
            # proj groups drip through the Ti=1 segs
            attention_seg(0, 0, [n2[0], f_v03[0], n2[1], f_v03[1], n2[2],
                                 f_v03[2], n2[3], f_v03[3], f_v03[4]], 1)
            attention_seg(0, 1, f_qk(3), 2)
            attention_seg(0, 2, f_v[0:4], 2)
            attention_seg(0, 3, f_v[4:8], 2)
            attention_seg(1, 0, f_p0[0:3], 4)
            attention_seg(1, 1, f_p0[3:6], 4)
            attention_seg(1, 2, f_p0[6:8], 8)
            # proj for the second T-half drips into the last seg's normalize
            # quarters (each 256-col quarter unlocks two 128-row t-blocks)
            f_p1 = [lambda t=t: proj_group(t, "act") for t in range(8, 16)]
            attention_seg(1, 3, [], 4, last=True, post=f_p1)
            nc.sync.dma_start(wrm_d[:], wrm[:])

    nc.compile()
    return nc


def _shard(x, Wq, bq, Wk, bk, Wv, bv, Wp, bp):
    import ml_dtypes
    f32 = np.float32
    bf16 = ml_dtypes.bfloat16
    fp8 = ml_dtypes.float8_e4m3fn
    mask01 = np.triu(np.ones((128, 128), f32)).astype(bf16)

    def ptile(w):  # [D, GD] -> [128, KT, GD] partition-contiguous
        return np.ascontiguousarray(
            w.reshape(KT, 128, GD).transpose(1, 0, 2))

    in_maps = []
    for c in range(N_CORES):
        b, g = divmod(c, HPC)
        sl = slice(GD * g, GD * (g + 1))
        xTb = np.ascontiguousarray(x[b].T)
        # [D, T] -> [128, 4, KT, 512] chunk-major fp8
        x8b = np.ascontiguousarray(
            xTb.astype(fp8).reshape(KT, 128, 4, 512).transpose(1, 2, 0, 3))
        in_maps.append({
            "xT": xTb.astype(bf16),
            "x8": x8b,
            "wqT": ptile((WS * Wq[sl, :].T).astype(fp8)),
            "wkT": ptile((WS * Wk[sl, :].T).astype(fp8)),
            "wvT": ptile(Wv[sl, :].T.astype(bf16)),
            "wpT": np.ascontiguousarray(Wp[:, sl].T, dtype=f32),
            "bq2": np.ascontiguousarray(
                WS * bq[sl].reshape(2, 128).T, dtype=f32),
            "bk2": np.ascontiguousarray(
                WS * bk[sl].reshape(2, 128).T, dtype=f32),
            "bvb": np.broadcast_to(bv[sl], (128, GD)).astype(f32),
            "mask01": mask01,
        })
    return in_maps


def run(inputs, trace=False):
    """Run the SPMD kernel; returns (output [B,T,D] f32, BassKernelResults)."""
    if "nc" not in _cache:
        _cache["nc"] = _build()
    nc = _cache["nc"]
    in_maps = _shard(**inputs)
    if trace:
        _install_ntff_hook()
    res = bass_utils.run_bass_kernel_spmd(
        nc, in_maps, core_ids=list(range(N_CORES)), trace=trace,
    )
    bp = np.asarray(inputs["bp"], dtype=np.float32)
    out = np.empty((B, T, D), dtype=np.float32)
    for b in range(B):
        acc = res.results[4 * b]["out"].astype(np.float32)
        for g in range(1, HPC):
            acc = acc + res.results[4 * b + g]["out"]
        out[b] = acc + bp
    return out, res


def kernel(**inputs):
    out, _ = run(inputs, trace=False)
    return out


def _install_ntff_hook():
    """antenv.axon_hooks is absent on this image; inject it so
    run_bass_kernel_spmd(trace=True) can capture NTFF profiles."""
    import sys, types
    if "antenv.axon_hooks" in sys.modules:
        return
    try:
        mod = types.ModuleType("antenv.axon_hooks")
        mod._hook = None
        mod.set_axon_ntff_profile_hook = lambda h: setattr(mod, "_hook", h)
        mod.get_axon_ntff_profile_hook = lambda: mod._hook
        sys.modules["antenv.axon_hooks"] = mod
        import antenv
        antenv.axon_hooks = mod
        from trn_agent_boot.trn_boot import _ntff_profile_via_ctypes
        mod.set_axon_ntff_profile_hook(
            _ntff_profile_via_ctypes("/opt/axon/libaxon_pjrt.so"))
    except Exception:
        pass


# revision 51
# speedup vs baseline: 1.2111x; 1.0186x over previous
"""Causal self-attention (B=2, T=2048, D=1024, H=16, Dh=64) on 8 TRN2 cores.

Sharding: core c = 4*b + g -> batch b (data parallel), head group g of 4
heads (tensor parallel on heads for Wq/Wk/Wv, column-split of the proj
input with the resulting partial-sum reduction done host-side at unshard).

Per-core dataflow (layouts chosen so no on-device transposes are needed):
  qT,kT [256, 2048] bf16 = W{q,k}_g @ x.T  (fp8 DoubleRow projections; x8
  is shipped pre-cast fp8 from the host so no on-device casts are needed.
  Note: on HW, DR costs 1 cy/output-col like bf16 -- its win is the 2x
  contraction depth per instruction, NOT a faster column rate.)
  v     [t-block 128, 4 heads x (64 v | 64 ones)] bf16
  attention, transposed: PT[tk, tq] = kT_h.T @ qT_h (bf16), exp on
  ACT -> bf16, causal mask as post-exp 0/1 multiply on GPSIMD,
  AV: yT[d, tq] + softmax column sums free via the ones columns of v
  normalize: yT * recip(sums) -> ytsb [256, 2048] f32r (proj lhsT layout)
  proj partial: out[t, :] = ytsb.T-block @ Wp_gT  (f32r)
Host: out[b] = sum_g partial[4b+g] + bp.

Segment order runs all Ti=0 (first T-half) segs for the 4 heads first, so
the first half of the output projection (and its out-DMA traffic) spreads
over the Ti=1 segs instead of bunching at the tail.
"""

import numpy as np

import concourse.bass as bass
import concourse.mybir as mybir
import concourse.tile as tile
from concourse import bacc
from concourse import bass_utils

F32 = mybir.dt.float32
F32R = mybir.dt.float32r
BF16 = mybir.dt.bfloat16
FP8 = mybir.dt.float8e4
DR = mybir.MatmulPerfMode.DoubleRow
WS = 16.0          # host scale on Wq/Wk (and bq/bk) so fp8 keeps mantissa

B, T, D = 2, 2048, 1024
H, DH = 16, 64
N_CORES = 8
HPC = 4            # heads per core
GD = HPC * DH      # 256 feature cols per core
KT = D // 128      # 8 k-tiles over the model dim
TB = T // 128      # 16 t-blocks of 128
SCL = 0.125        # logit scale 1/sqrt(Dh)

_cache = {}


def _build():
    nc = bacc.Bacc("TRN2", target_bir_lowering=False, debug=False,
                   num_devices=N_CORES)

    xT_d = nc.dram_tensor("xT", [D, T], BF16, kind="ExternalInput")
    # x8: host-cast fp8 copy of x, chunk-major [128, 4, KT, 512] so each
    # 512-col chunk is one contiguous-per-partition DMA
    x8_d = nc.dram_tensor("x8", [128, 4, KT, 512], FP8, kind="ExternalInput")
    wqT_d = nc.dram_tensor("wqT", [128, KT, GD], FP8, kind="ExternalInput")
    wkT_d = nc.dram_tensor("wkT", [128, KT, GD], FP8, kind="ExternalInput")
    wvT_d = nc.dram_tensor("wvT", [128, KT, GD], BF16, kind="ExternalInput")
    wpT_d = nc.dram_tensor("wpT", [GD, D], F32R, kind="ExternalInput")
    bq_d = nc.dram_tensor("bq2", [128, 2], F32, kind="ExternalInput")
    bk_d = nc.dram_tensor("bk2", [128, 2], F32, kind="ExternalInput")
    bvb_d = nc.dram_tensor("bvb", [128, GD], F32, kind="ExternalInput")
    msk_d = nc.dram_tensor("mask01", [128, 128], BF16, kind="ExternalInput")
    out_d = nc.dram_tensor("out", [T, D], BF16, kind="ExternalOutput")
    wrm_d = nc.dram_tensor("wrm", [128, 1], F32, kind="ExternalOutput")

    with tile.TileContext(nc) as tc:
        with (
            tc.tile_pool(name="const", bufs=1) as cp,
            tc.tile_pool(name="big", bufs=1) as bp_,
            tc.tile_pool(name="work", bufs=4) as wp_,
            tc.tile_pool(name="outp", bufs=6) as op_,
            tc.tile_pool(name="pA", bufs=2, space="PSUM") as pA,
            tc.tile_pool(name="pB", bufs=1, space="PSUM") as pB,
            tc.tile_pool(name="pC", bufs=2, space="PSUM") as pC,
        ):
            # ---- loads, ordered by need-time. First matmul needs wq + x8
            # chunk 0 only; queue issue cost (~0.6us each) is the real head
            # constraint, so big tensors ride single DMAs. ----
            wq = cp.tile([128, KT, GD], FP8, tag="wq", name="wq")
            wk = cp.tile([128, KT, GD], FP8, tag="wk", name="wk")
            wv = cp.tile([128, KT, GD], BF16, tag="wv", name="wv")
            bq2 = cp.tile([128, 2], F32, tag="bq2", name="bq2")
            bk2 = cp.tile([128, 2], F32, tag="bk2", name="bk2")
            bvb = cp.tile([128, GD], F32, tag="bvb", name="bvb")
            msk = cp.tile([128, 128], BF16, tag="msk", name="msk")
            x8 = cp.tile([128, 4, KT, 512], FP8, tag="x8", name="x8")
            xt = [cp.tile([128, T], BF16, tag=f"xt{k}", name=f"xt{k}")
                  for k in range(KT)]

            # gpsimd issues no loads: its queue must stay clear for the
            # exp->mask->AV critical chain
            # first matmul needs only wq k-tiles 0:2 + x8 chunk-0 k-tiles
            # 0:2; split those loads so transfer time off the critical path
            nc.scalar.dma_start(wq[:, 0:2], wqT_d[:, 0:2])
            nc.sync.dma_start(x8[:, 0, 0:2], x8_d[:, 0, 0:2])
            nc.scalar.dma_start(wq[:, 2:4], wqT_d[:, 2:4])
            nc.sync.dma_start(x8[:, 0, 2:4], x8_d[:, 0, 2:4])
            nc.scalar.dma_start(wq[:, 4:KT], wqT_d[:, 4:KT])
            nc.sync.dma_start(x8[:, 0, 4:KT], x8_d[:, 0, 4:KT])
            nc.scalar.dma_start(bq2[:], bq_d[:])
            # dummy exp during the head: pulls the ACT table load (~1.3us)
            # off the first attention block's critical path
            wrmup = cp.tile([128, 2], F32, tag="wrmup", name="wrmup")
            nc.scalar.activation(wrmup[:], bq2[:],
                                 mybir.ActivationFunctionType.Exp,
                                 scale=1.0)
            nc.sync.dma_start(wk[:], wkT_d[:])
            nc.scalar.dma_start(bk2[:], bk_d[:])
            nc.sync.dma_start(x8[:, 1, 0:4], x8_d[:, 1, 0:4])
            nc.sync.dma_start(x8[:, 1, 4:KT], x8_d[:, 1, 4:KT])
            nc.scalar.dma_start(x8[:, 2], x8_d[:, 2])
            nc.scalar.dma_start(x8[:, 3], x8_d[:, 3])
            nc.sync.dma_start(msk[:], msk_d[:])
            nc.sync.dma_start(wv[:], wvT_d[:])
            nc.sync.dma_start(bvb[:], bvb_d[:])
            # xt rows: needed by v_groups (first ~12us in); one DMA per row
            for k in range(KT):
                (nc.sync if k % 2 == 0 else nc.scalar).dma_start(
                    xt[k][:], xT_d[k * 128:(k + 1) * 128, :])
            wpt = []
            for p in range(2):
                t_ = cp.tile([128, D], F32R, tag=f"wp{p}", name=f"wp{p}")
                nc.scalar.dma_start(t_[:], wpT_d[p * 128:(p + 1) * 128, :])
                wpt.append(t_)

            qt = [bp_.tile([128, T], BF16, tag=f"qt{m}", name=f"qt{m}")
                  for m in range(2)]
            kt = [bp_.tile([128, T], BF16, tag=f"kt{m}", name=f"kt{m}")
                  for m in range(2)]
            ytsb = [bp_.tile([128, T], F32R, tag=f"yt{p}", name=f"yt{p}")
                    for p in range(2)]
            vt = [bp_.tile([128, 4, 2, DH], BF16, tag=f"v{t}", name=f"v{t}")
                  for t in range(TB)]

            def qk_group(dst, w, b2, m, n):
                # fp8 DoubleRow: two k-tiles per matmul (lhsT [128,2,128],
                # rhs [128,2,512]) -- on HW, DR costs 1 cy per output col
                # like bf16, but packs 2x contraction depth per instruction,
                # so 4 full-width matmuls cover all 8 k-tiles.
                # Values carry a x16 host scale; exp scale compensates.
                ps = pC.tile([128, 512], F32, tag=pC.name, name="psqk")
                for kp in range(4):
                    nc.tensor.matmul(
                        ps[:],
                        w[:, 2 * kp:2 * kp + 2, m * 128:(m + 1) * 128],
                        x8[:, n, 2 * kp:2 * kp + 2, :],
                        start=(kp == 0), stop=(kp == 3),
                        perf_mode=DR,
                    )
                nc.vector.tensor_scalar_add(
                    dst[m][:, n * 512:(n + 1) * 512], ps[:], b2[:, m:m + 1],
                )

            def v_group(t):
                ps = pC.tile([128, 512], F32, tag=pC.name, name="psv")
                for k in range(KT):
                    nc.tensor.matmul(
                        ps[:, 0:GD],
                        xt[k][:, t * 128:(t + 1) * 128],
                        wv[:, k, :],
                        start=(k == 0), stop=(k == KT - 1),
                    )
                nc.vector.tensor_add(
                    vt[t][:, :, 0, :],
                    ps[:, 0:GD].rearrange("p (h d) -> p h d", h=4),
                    bvb.rearrange("p (h d) -> p h d", h=4),
                )

            def proj_group(t, copy_eng):
                # each 512-col half is copied then DMA'd out immediately so
                # the final-output tail pipelines at half-tile granularity.
                # Late groups (copy_eng == "act") run during the tail: the
                # PSUM->SBUF copies go to the then-idle scalar engine, and
                # the out-DMAs split across queues so the last transfers
                # ride several DMA engines instead of one.
                ob = op_.tile([128, 1024], BF16, tag="ob", name="ob")
                for n in range(2):
                    po = pC.tile([128, 512], F32, tag=pC.name, name="pso")
                    for p in range(2):
                        nc.tensor.matmul(
                            po[:],
                            ytsb[p][:, 128 * t:128 * (t + 1)],
                            wpt[p][:, 512 * n:512 * (n + 1)],
                            start=(p == 0), stop=(p == 1),
                        )
                    if copy_eng == "act":
                        nc.scalar.copy(ob[:, 512 * n:512 * (n + 1)], po[:])
                        for h_ in range(2):
                            cs = slice(512 * n + 256 * h_,
                                       512 * n + 256 * (h_ + 1))
                            eng = (nc.sync, nc.scalar,
                                   nc.gpsimd)[(2 * t + 2 * n + h_) % 3]
                            eng.dma_start(
                                out_d[128 * t:128 * (t + 1), cs], ob[:, cs])
                    else:
                        nc.vector.tensor_copy(
                            ob[:, 512 * n:512 * (n + 1)], po[:])
                        nc.sync.dma_start(
                            out_d[128 * t:128 * (t + 1),
                                  512 * n:512 * (n + 1)],
                            ob[:, 512 * n:512 * (n + 1)])

            def attention_seg(Ti, h, fillers, every, last=False,
                              post=None):
                hp, j = h // 2, h % 2
                ytp = pB.tile([128, 1024], F32, tag=pB.name, name="psyt")
                nblk = 8 * (Ti + 1)
                SKEW = 3       # AV trails QK/exp: the PE never
                pend = []      # waits on an exp that was just issued
                def do_av(tkb, ptsb):
                    s = max(0, 128 * tkb - 1024 * Ti)
                    for bk in range(2):
                        c0, c1 = max(s, 512 * bk), 512 * (bk + 1)
                        if c0 >= c1:
                            continue
                        nc.tensor.matmul(
                            ytp[:, c0:c1],
                            vt[tkb][:, h, :, :].rearrange("p a d -> p (a d)"),
                            ptsb[:, c0:c1],
                            start=(tkb == 0), stop=(tkb == nblk - 1),
                        )
                for tkb in range(nblk + SKEW):
                    if tkb < nblk:
                        s = max(0, 128 * tkb - 1024 * Ti)
                        pt = pA.tile([128, 1024], F32, tag=pA.name,
                                     name="pspt")
                        for bk in range(2):
                            c0, c1 = max(s, 512 * bk), 512 * (bk + 1)
                            if c0 >= c1:
                                continue
                            nc.tensor.matmul(
                                pt[:, c0:c1],
                                kt[hp][64 * j:64 * j + 64,
                                       128 * tkb:128 * (tkb + 1)],
                                qt[hp][64 * j:64 * j + 64,
                                       1024 * Ti + c0:1024 * Ti + c1],
                                start=True, stop=True,
                            )
                        ptsb = wp_.tile([128, 1024], BF16, tag="ptsb",
                                        name="ptsb", bufs=6)
                        nc.scalar.activation(
                            ptsb[:, s:1024], pt[:, s:1024],
                            mybir.ActivationFunctionType.Exp,
                            scale=SCL / (WS * WS),
                        )
                        if 128 * tkb >= 1024 * Ti:  # diagonal block
                            nc.gpsimd.tensor_mul(
                                ptsb[:, s:s + 128], ptsb[:, s:s + 128],
                                msk[:],
                            )
                        pend.append((tkb, ptsb))
                    if tkb >= SKEW:
                        do_av(*pend.pop(0))
                    if fillers and tkb % every == every - 1:
                        fillers.pop(0)()
                while pend:
                    do_av(*pend.pop(0))
                # free the PSUM accumulator promptly; 1/sums via the custom
                # fast-approx DVE reciprocal (~5x cheaper than InstReciprocal,
                # 18 good bits -- plenty for softmax normalization)
                if not last:
                    src = wp_.tile([128, 1024], F32, tag="ysb", name="ysb")
                    nc.vector.tensor_copy(src[:], ytp[:])
                else:
                    src = ytp
                # custom-DVE fast reciprocal: in/out must share a partition
                # base (offset-crossing APs feed it garbage -> NaN), and
                # tensor_tensor needs equal input bases -- so recip at base
                # 64, then a single-input gpsimd copy crosses down to base 0
                rc = wp_.tile([128, 1024], F32, tag="recip", name="recip")
                cp_eng = nc.gpsimd if last else nc.vector
                for q in range(4):
                    cs = slice(256 * q, 256 * (q + 1))
                    nc.vector.reciprocal_approx_fast(rc[:, cs], src[:, cs])
                    cp_eng.tensor_copy(rc[0:64, cs], rc[64:128, cs])
                    nc.vector.tensor_mul(
                        ytsb[hp][64 * j:64 * j + 64,
                                 1024 * Ti + 256 * q:1024 * Ti + 256 * (q + 1)],
                        src[0:64, cs], rc[0:64, cs],
                    )
                    if post:
                        post.pop(0)()
                        post.pop(0)()

            # ---- schedule ----
            # all ones-column memsets up front: a memset inside v_group
            # lands on the gpsimd queue behind mask-multiplies that wait on
            # exp, chaining the v fillers (and the PE) to the exp pace
            for t in range(TB):
                nc.gpsimd.memset(vt[t][:, :, 1, :], 1.0)
            # head clock pre-ramp: the PE idles ~10us while loads land, then
            # pays ~3.5us of half-clock p-state ramp on its first real
            # matmuls. Dummy matmuls on the memset-only vt[0] (ready ~1us,
            # no DMA dependency) are interleaved BETWEEN the projection
            # groups: if loads are late they fill the stalls and hold the
            # clock; if loads are early each group is delayed by at most
            # two dummies (~0.4us). All accumulate into one pB tile (free
            # until seg 0) so the pC rotation is untouched; a reduce_max
            # into a debug output prevents elimination.
            wrm = bp_.tile([128, 1], F32, tag="wrm", name="wrm")
            wps = pB.tile([128, 512], F32, tag=pB.name, name="pswrm")
            wct = [0]

            def warm(k, stop=False):
                for i in range(k):
                    nc.tensor.matmul(
                        wps[:],
                        vt[0][:, 0, :, :].rearrange("p a d -> p (a d)"),
                        vt[0].rearrange("p h a d -> p (h a d)"),
                        start=(wct[0] == 0),
                        stop=(stop and i == k - 1),
                    )
                    wct[0] += 1

            # q/k projections for the first T-half (n=0,1) of BOTH m-chunks
            # up front: every Ti=0 seg only touches q/k columns 0:1024.
            # The three head-start v groups (seg 0's SKEW lead) sit between
            # the n=0 and n=1 groups to cover the x8 chunk-1 transfer.
            warm(8)
            for m in range(2):
                qk_group(qt, wq, bq2, m, 0)
                warm(2)
                qk_group(kt, wk, bk2, m, 0)
                warm(2)
            for t in range(3):
                v_group(t)
                warm(1)
            for m in range(2):
                qk_group(qt, wq, bq2, m, 1)
                warm(2)
                qk_group(kt, wk, bk2, m, 1)
                warm(2 if m == 0 else 0)
            warm(1, stop=True)
            nc.vector.reduce_max(wrm[:], wps[:], axis=mybir.AxisListType.X)

            def f_qk(n):  # remaining projection column chunks
                return [lambda m=m, w=w, b=b, d=d: qk_group(d, w, b, m, n)
                        for m in range(2) for (d, w, b) in
                        [(qt, wq, bq2), (kt, wk, bk2)]]
            f_v03 = [lambda t=t: v_group(t) for t in range(3, 8)]
            f_v = [lambda t=t: v_group(t) for t in range(8, 16)]
            f_p0 = [lambda t=t: proj_group(t, "vec") for t in range(8)]

            n2 = f_qk(2)
            # Ti=0 for all heads first, then Ti=1; the first output half's
            # proj groups drip through the Ti=1 segs
            attention_seg(0, 0, [n2[0], f_v03[0], n2[1], f_v03[1], n2[2],
                                 f_v03[2], n2[3], f_v03[3], f_v03[4]], 1)
            attention_seg(0, 1, f_qk(3), 2)
            attention_seg(0, 2, f_v[0:4], 2)
            attention_seg(0, 3, f_v[4:8], 2)
            attention_seg(1, 0, f_p0[0:3], 4)
            attention_seg(1, 1, f_p0[3:6], 4)
            attention_seg(1, 2, f_p0[6:8], 8)
            # proj for the second T-half drips into the last seg's normalize
            # quarters (each 256-col quarter unlocks two 128-row t-blocks)
            f_p1 = [lambda t=t: proj_group(t, "act") for t in range(8, 16)]
            attention_seg(1, 3, [], 4, last=True, post=f_p1)
            nc.sync.dma_start(wrm_d[:], wrm[:])

    nc.compile()
    return nc


def _shard(x, Wq, bq, Wk, bk, Wv, bv, Wp, bp):
    import ml_dtypes
    f32 = np.float32
    bf16 = ml_dtypes.bfloat16
    fp8 = ml_dtypes.float8_e4m3fn
    mask01 = np.triu(np.ones((128, 128), f32)).astype(bf16)

    def ptile(w):  # [D, GD] -> [128, KT, GD] partition-contiguous
        return np.ascontiguousarray(
            w.reshape(KT, 128, GD).transpose(1, 0, 2))

    in_maps = []
    for c in range(N_CORES):
        b, g = divmod(c, HPC)
        sl = slice(GD * g, GD * (g + 1))
        xTb = np.ascontiguousarray(x[b].T)
        # [D, T] -> [128, 4, KT, 512] chunk-major fp8
        x8b = np.ascontiguousarray(
            xTb.astype(fp8).reshape(KT, 128, 4, 512).transpose(1, 2, 0, 3))
        in_maps.append({
            "xT": xTb.astype(bf16),
            "x8": x8b,
            "wqT": ptile((WS * Wq[sl, :].T).astype(fp8)),
            "wkT": ptile((WS * Wk[sl, :].T).astype(fp8)),
            "wvT": ptile(Wv[sl, :].T.astype(bf16)),
            "wpT": np.ascontiguousarray(Wp[:, sl].T, dtype=f32),
            "bq2": np.ascontiguousarray(
                WS * bq[sl].reshape(2, 128).T, dtype=f32),
            "bk2": np.ascontiguousarray(
                WS * bk[sl].reshape(2, 128).T, dtype=f32),
            "bvb": np.broadcast_to(bv[sl], (128, GD)).astype(f32),
            "mask01": mask01,
        })
    return in_maps


def run(inputs, trace=False):
    """Run the SPMD kernel; returns (output [B,T,D] f32, BassKernelResults)."""
    if "nc" not in _cache:
        _cache["nc"] = _build()
    nc = _cache["nc"]
    in_maps = _shard(**inputs)
    if trace:
        _install_ntff_hook()
    res = bass_utils.run_bass_kernel_spmd(
        nc, in_maps, core_ids=list(range(N_CORES)), trace=trace,
    )
    bp = np.asarray(inputs["bp"], dtype=np.float32)
    out = np.empty((B, T, D), dtype=np.float32)
    for b in range(B):
        acc = res.results[4 * b]["out"].astype(np.float32)
        for g in range(1, HPC):
            acc = acc + res.results[4 * b + g]["out"]
        out[b] = acc + bp
    return out, res


def kernel(**inputs):
    out, _ = run(inputs, trace=False)
    return out


def _install_ntff_hook():
    """antenv.axon_hooks is absent on this image; inject it so
    run_bass_kernel_spmd(trace=True) can capture NTFF profiles."""
    import sys, types
    if "antenv.axon_hooks" in sys.modules:
        return
    try:
        mod = types.ModuleType("antenv.axon_hooks")
        mod._hook = None
        mod.set_axon_ntff_profile_hook = lambda h: setattr(mod, "_hook", h)
        mod.get_axon_ntff_profile_hook = lambda: mod._hook
        sys.modules["antenv.axon_hooks"] = mod
        import antenv
        antenv.axon_hooks = mod
        from trn_agent_boot.trn_boot import _ntff_profile_via_ctypes
        mod.set_axon_ntff_profile_hook(
            _ntff_profile_via_ctypes("/opt/axon/libaxon_pjrt.so"))
    except Exception:
        pass
